# revision 4
# baseline (speedup 1.0000x reference)
"""GQA causal attention with RoPE, distributed over 8 trn2 NeuronCores.

Sharding: 4-way data parallel over batch x 2-way tensor parallel over heads.
Core c = 2*b + t handles batch b with query heads [t*8, (t+1)*8) and KV heads
[t*2, (t+1)*2).  Each core computes a row-sharded out_proj partial; the pair
partials are summed on the host during unsharding.

On-chip algorithm (per core, bf16 matmuls / fp32 softmax):
  1. QKV projections from host-prearranged partition-major inputs
     (xT/wqkv/wo stored as [128, chunks*cols] so every DMA descriptor is a
     full 2-chunk 4-6KB partition line; the first chunk is split across
     4 partition-group DMAs so the first matmul starts ~1.5us after the
     queues open).  Token chunks m=0,1 and m=2's q-columns are projected
     k-outer while the input streams; the PSUM pools are split
     (q [P,1024] bufs=3 / kv [P,512] bufs=2) so three chunks are in
     flight during the DMA ramp.
  2. Each projection chunk is staged PSUM->SBUF (bf16) on ScalarE; RoPE
     runs all-bf16 on the DVE (2x perf mode).  The q transposes are issued
     on the ScalarE HWDGE ring and the k transposes on the sync ring so
     they never FIFO behind the bulk input stream, and the last chunk
     projects its q columns first so phase B's qT dependency lands before
     the early score pieces run out.
  3. Scores computed TRANSPOSED (scoresT[k_tok, q_tok]) so no probs
     transpose is needed: exp on ScalarE, column sums accumulated on DVE,
     summed across partitions by a ones-matmul, AV matmul consumes probsT
     directly, normalization happens once on the attention output.
  4. Causality: blocks with ki > qi are never computed; the diagonal block
     is masked with a precomputed upper-triangular 0/1 mask after exp.
     Heads are processed in PAIRS with interleaved k-chunks so the
     score->exp->AV chain of one head hides under the other's matmuls
     (phase B paces on ScalarE exp throughput).
  5. out_proj from feature-major attnT with wo as the moving operand,
     fine-grained (one 512-col PSUM bank per block, k-inner accumulation).
     The last chunk's output stores are split across partition groups /
     queues so the tail after the final matmul is ~1.5us.
"""

import math
import sys

sys.path.insert(0, "/opt/trn_rl_repo")

import ml_dtypes
import numpy as np

import concourse.bacc as bacc
import concourse.mybir as mybir
import concourse.tile as tile
from concourse.bass import _add_dep_helper
from concourse.bass_utils import run_bass_kernel_spmd
from concourse.masks import make_upper_triangular

B, S, HID = 4, 1024, 2048
H, KV, D = 16, 4, 128
P = 128
TP = 2                  # tensor-parallel ways (head split)
HL = H // TP            # 8 query heads per core
KVL = KV // TP          # 2 kv heads per core
QD = HL * D             # 1024
KD = KVL * D            # 256
SC = S // P             # 8 token chunks
KC = HID // P           # 16 hidden chunks
QKVD = QD + 2 * KD      # 1536 = q 1024 | k 256 | v 256
NCORES = 8
BF = mybir.dt.bfloat16
F32 = mybir.dt.float32
Exp = mybir.ActivationFunctionType.Exp

_NC_CACHE = {}


def _ensure_ntff_hook():
    """The agent image's antenv lacks axon_hooks, so bass_utils' trace=True
    path can't find the NTFF profile hook trn_boot would have registered.
    Recreate the module and register the ctypes-based hook ourselves."""
    try:
        from antenv.axon_hooks import get_axon_ntff_profile_hook  # noqa: F401
        return
    except ImportError:
        pass
    import types

    import antenv

    mod = types.ModuleType("antenv.axon_hooks")
    _state = {"hook": None}
    mod.set_axon_ntff_profile_hook = lambda h: _state.__setitem__("hook", h)
    mod.get_axon_ntff_profile_hook = lambda: _state["hook"]
    sys.modules["antenv.axon_hooks"] = mod
    antenv.axon_hooks = mod
    try:
        from trn_agent_boot.trn_boot import _ntff_profile_via_ctypes

        hook = _ntff_profile_via_ctypes("/opt/axon/libaxon_pjrt.so")
        if hook is not None:
            mod.set_axon_ntff_profile_hook(hook)
    except Exception as e:  # pragma: no cover
        print(f"NTFF hook registration failed: {e}", file=sys.stderr)


def _pieces(start, end, step=512):
    """Split [start, end) into spans of at most `step`, aligned so no span
    crosses a `step` boundary (PSUM: one bank per matmul)."""
    out = []
    a = start
    while a < end:
        b = min((a // step + 1) * step, end)
        out.append((a, b))
        a = b
    return out


def build_nc():
    nc = bacc.Bacc("TRN2", target_bir_lowering=False, debug=False,
                   num_devices=NCORES)

    # partition-major DRAM layouts: row p holds chunk-c data at [c*cols ...]
    xT = nc.declare_dram_parameter("xT", [P, KC * S], BF, isOutput=False)
    wqkv = nc.declare_dram_parameter("wqkv", [P, KC * QKVD], BF, isOutput=False)
    wo = nc.declare_dram_parameter("wo", [P, HL * HID], BF, isOutput=False)
    cos_t = nc.declare_dram_parameter("cos_t", [P, SC * D], BF, isOutput=False)
    sin_t = nc.declare_dram_parameter("sin_t", [P, SC * D], BF, isOutput=False)
    out = nc.declare_dram_parameter("out", [S, HID], BF, isOutput=True)

    with tile.TileContext(nc) as tc:
        # ---- persistent pools (allocated first: fixed addresses) ----
        cpool = tc.alloc_tile_pool(name="consts", bufs=1)
        wpool = tc.alloc_tile_pool(name="wpool", bufs=1)
        qkvpool = tc.alloc_tile_pool(name="qkvpool", bufs=1)
        # phase B's SBUF pool allocated BEFORE phase A's pools so their
        # address ranges are disjoint: no release barrier between A and B.
        battn = tc.alloc_tile_pool(name="battn", bufs=2)

        utmask = cpool.tile([P, P], BF)
        ones_mat = cpool.tile([P, P], BF)

        sb_wo = wpool.tile([P, HL, HID], BF)

        sb_qT = qkvpool.tile([P, HL, S], BF)      # feature-major q
        sb_kT = qkvpool.tile([P, KVL, S], BF)     # feature-major k
        sb_v = qkvpool.tile([P, SC, KD], BF)      # token-major v
        sb_attnT = qkvpool.tile([P, HL, S], BF)   # feature-major attn out

        # ---------------- Phase A: projections + RoPE -----------------
        projpool = tc.alloc_tile_pool(name="proj", bufs=1)
        ropepool = tc.alloc_tile_pool(name="rope", bufs=2)
        # split projection PSUM: q-part [P,1024] (2 banks) x3, kv-part
        # [P,512] (1 bank) x2 -> 8 banks, three chunks in flight at ramp
        ps_qq = tc.alloc_tile_pool(name="ps_qq", bufs=3, space="PSUM")
        ps_kv = tc.alloc_tile_pool(name="ps_kv", bufs=2, space="PSUM")

        sb_xT = projpool.tile([P, KC, S], BF)
        sb_wqkv = projpool.tile([P, KC, QKVD], BF)

        # chunk 0 fine-split across partition groups (parallel queues) so
        # the first matmul waits on ~32 descriptors, not 128
        for pg in range(4):
            ps_, pe_ = pg * 32, (pg + 1) * 32
            nc.sync.dma_start(out=sb_wqkv[ps_:pe_, 0, 0:512],
                              in_=wqkv[ps_:pe_, 0:512])
        for pg in range(4):
            ps_, pe_ = pg * 32, (pg + 1) * 32
            nc.sync.dma_start(out=sb_xT[ps_:pe_, 0, 0:384],
                              in_=xT[ps_:pe_, 0:384])
        for pg in range(2):
            ps_, pe_ = pg * 64, (pg + 1) * 64
            nc.sync.dma_start(out=sb_wqkv[ps_:pe_, 0, 512:QKVD],
                              in_=wqkv[ps_:pe_, 512:QKVD])
        for pg in range(2):
            ps_, pe_ = pg * 64, (pg + 1) * 64
            nc.sync.dma_start(out=sb_xT[ps_:pe_, 0, 384:S],
                              in_=xT[ps_:pe_, 384:S])
        nc.sync.dma_start(out=sb_wqkv[:, 1, :],
                          in_=wqkv[:, QKVD:2 * QKVD])
        nc.sync.dma_start(out=sb_xT[:, 1, :], in_=xT[:, S:2 * S])
        sb_ck = projpool.tile([P, SC, D], BF)
        nc.sync.dma_start(out=sb_ck[:, :, :], in_=cos_t[:, :])
        sb_sk = projpool.tile([P, SC, D], BF)
        nc.sync.dma_start(out=sb_sk[:, :, :], in_=sin_t[:, :])
        # chunk pairs: one descriptor per partition line (4-6KB each)
        for c in range(2, KC, 2):
            nc.sync.dma_start(out=sb_wqkv[:, c:c + 2, :],
                              in_=wqkv[:, c * QKVD:(c + 2) * QKVD])
            nc.sync.dma_start(out=sb_xT[:, c:c + 2, :],
                              in_=xT[:, c * S:(c + 2) * S])
        # wo is only needed in phase C: delay its load until the input
        # streaming has drained (dep added below); 2-chunk pieces (8KB lines)
        wo_dmas = []
        for c in range(0, HL, 2):
            wo_dmas.append(nc.sync.dma_start(
                out=sb_wo[:, c:c + 2, :],
                in_=wo[:, c * HID:(c + 2) * HID]))

        # mask/ones builders issued after the DMA starts so the sync engine
        # kicks off the input stream first (they are not needed until B)
        make_upper_triangular(nc, utmask[:, :], val=1.0, diag=True)
        nc.vector.memset(ones_mat[:, :], 1.0)

        HALF = D // 2

        def rope_block(sb_src, lo, nh, m):
            """RoPE `nh` consecutive heads of the staged bf16 chunk (cols
            [lo, lo+nh*D)) in one batched op per step, via free-dim-broadcast
            cos/sin APs.  All-bf16 so the DVE runs in 2x mode.  Returns a
            bf16 SBUF tile [P, nh*D]."""
            t1 = ropepool.tile([P, nh, D], BF, tag="t1")
            ro = ropepool.tile([P, nh * D], BF, tag="ro", bufs=3)
            src = sb_src[:, lo:lo + nh * D].rearrange("p (h d) -> p h d", h=nh)
            sin_lo = sb_sk[:, m:m + 1, 0:HALF].broadcast_to([P, nh, HALF])
            sin_hi = sb_sk[:, m:m + 1, HALF:D].broadcast_to([P, nh, HALF])
            cos_b = sb_ck[:, m:m + 1, :].broadcast_to([P, nh, D])
            # rot_half * sin (sin table pre-negated on first half)
            nc.vector.tensor_mul(t1[:, :, 0:HALF], src[:, :, HALF:D], sin_lo)
            nc.vector.tensor_mul(t1[:, :, HALF:D], src[:, :, 0:HALF], sin_hi)
            ror = ro[:, :].rearrange("p (h d) -> p h d", h=nh)
            # ro = src*cos + t1
            nc.vector.tensor_mul(ror, src, cos_b)
            nc.vector.tensor_add(ror, ror, t1[:, :, :])
            return ro

        def proj_q(pq, m, k):
            st, sp = (k == 0), (k == KC - 1)
            lhsT = sb_xT[:, k, m * P:(m + 1) * P]
            for n in (0, 1):
                mm = nc.tensor.matmul(
                    pq[:, n * 512:(n + 1) * 512], lhsT,
                    sb_wqkv[:, k, n * 512:(n + 1) * 512],
                    start=st, stop=sp)
            return mm

        def proj_kv(pkv, m, k):
            st, sp = (k == 0), (k == KC - 1)
            lhsT = sb_xT[:, k, m * P:(m + 1) * P]
            return nc.tensor.matmul(
                pkv[:, :], lhsT, sb_wqkv[:, k, 1024:QKVD],
                start=st, stop=sp)

        def stage_q(pq):
            sb_qk = ropepool.tile([P, QD + KD], BF, tag="qk")
            nc.scalar.copy(sb_qk[:, 0:512], pq[:, 0:512])
            nc.scalar.copy(sb_qk[:, 512:QD], pq[:, 512:QD])
            return sb_qk

        def stage_kv(sb_qk, pkv, m):
            nc.scalar.copy(sb_qk[:, QD:QD + KD], pkv[:, 0:KD])
            nc.scalar.copy(sb_v[:, m, :], pkv[:, KD:2 * KD])

        def rope_q(sb_qk, m):
            # q transposes ride the ScalarE (Act) HWDGE ring: never FIFO
            # behind the bulk input stream on the sync ring
            ms = slice(m * P, (m + 1) * P)
            q_ro1 = rope_block(sb_qk, 0, 4, m)
            nc.scalar.dma_start_transpose(out=sb_qT[:, 0:4, ms], in_=q_ro1[:, :])
            q_ro2 = rope_block(sb_qk, 4 * D, 4, m)
            nc.scalar.dma_start_transpose(out=sb_qT[:, 4:8, ms], in_=q_ro2[:, :])

        def rope_k(sb_qk, m):
            ms = slice(m * P, (m + 1) * P)
            k_ro = rope_block(sb_qk, QD, KVL, m)
            nc.sync.dma_start_transpose(out=sb_kT[:, :, ms], in_=k_ro[:, :])

        def finish_m(pq, pkv, m):
            sb_qk = stage_q(pq)
            stage_kv(sb_qk, pkv, m)
            rope_q(sb_qk, m)
            rope_k(sb_qk, m)

        # m=0, m=1 and m=2's q columns share each arriving k-chunk during
        # the DMA ramp (three chunks in flight across the split PSUM pools)
        pq0 = ps_qq.tile([P, QD], F32, tag="pq")
        pkv0 = ps_kv.tile([P, 2 * KD], F32, tag="pkv")
        pq1 = ps_qq.tile([P, QD], F32, tag="pq")
        pkv1 = ps_kv.tile([P, 2 * KD], F32, tag="pkv")
        pq2 = ps_qq.tile([P, QD], F32, tag="pq")
        for k in range(KC):
            proj_q(pq0, 0, k)
            proj_kv(pkv0, 0, k)
            proj_q(pq1, 1, k)
            proj_kv(pkv1, 1, k)
            mm = proj_q(pq2, 2, k)
        # release the wo load only once the input streaming has drained
        for wd in wo_dmas:
            _add_dep_helper(wd.ins, mm.ins,
                            reason="delay wo load past input ramp")
        finish_m(pq0, pkv0, 0)
        pkv2 = ps_kv.tile([P, 2 * KD], F32, tag="pkv")
        for k in range(KC):
            proj_kv(pkv2, 2, k)
        finish_m(pq1, pkv1, 1)
        prev = (pq2, pkv2, 2)
        for m in range(3, SC - 1):
            pq = ps_qq.tile([P, QD], F32, tag="pq")
            pkv = ps_kv.tile([P, 2 * KD], F32, tag="pkv")
            for k in range(KC):
                proj_q(pq, m, k)
                proj_kv(pkv, m, k)
            finish_m(*prev)
            prev = (pq, pkv, m)

        # Last chunk: q columns first so its qT (needed by phase B almost
        # immediately) lands while the PE still has kv-proj + early score
        # work; k/v columns follow and kT rides the sync ring in parallel.
        M7 = SC - 1
        pq7 = ps_qq.tile([P, QD], F32, tag="pq")
        for k in range(KC):
            proj_q(pq7, M7, k)
        finish_m(*prev)
        sb_qk7 = stage_q(pq7)
        rope_q(sb_qk7, M7)
        pkv7 = ps_kv.tile([P, 2 * KD], F32, tag="pkv")
        for k in range(KC):
            proj_kv(pkv7, M7, k)
        stage_kv(sb_qk7, pkv7, M7)
        ps_kv.release()
        ps_qq.release()
        # Shared 4-slot [P,512] fp32 PSUM pool for score tiles, ones-matmul
        # sum tiles AND phase C's out_proj tiles; 2-slot [P,S] pool for the
        # paired heads' AV accumulators.
        ps_small = tc.alloc_tile_pool(name="ps_small", bufs=4, space="PSUM")
        ps_av = tc.alloc_tile_pool(name="ps_av", bufs=2, space="PSUM")

        exp_scale = float(1 / math.sqrt(D))
        # ragged probsT: row ki only stores the causal columns [ki*P, S)
        OFF = [0]
        for ki in range(SC):
            OFF.append(OFF[-1] + S - ki * P)
        PTOT = OFF[-1]          # 4608

        head_tiles = {}

        def get_head_tiles(h):
            if h not in head_tiles:
                probsT = battn.tile([P, PTOT], BF, tag="probsT", bufs=3,
                                    name=f"probsT{h}")
                acc = battn.tile([P, S], BF, tag="acc", bufs=3,
                                 name=f"acc{h}")
                head_tiles[h] = (probsT, acc)
            return head_tiles[h]

        def pt(probsT, ki, a, b):
            q0 = ki * P
            return probsT[:, OFF[ki] + (a - q0):OFF[ki] + (b - q0)]

        # early score pieces: cover the PE while the last chunk's RoPE +
        # transposes drain (they only touch token chunks 0-3)
        early_done = set()
        for h in (0, 1, 2):
            probsT, _ = get_head_tiles(h)
            for ki in range(4):
                a = ki * P
                psc = ps_small.tile([P, 512], F32, tag="ps", name="psce")
                nc.tensor.matmul(psc[:, 0:512 - a], sb_kT[:, 0, a:a + P],
                                 sb_qT[:, h, a:512], start=True, stop=True)
                nc.scalar.activation(pt(probsT, ki, a, 512),
                                     psc[:, 0:512 - a],
                                     Exp, scale=exp_scale)
                early_done.add((h, ki, a))

        rope_k(sb_qk7, M7)
        ropepool.release()
        projpool.release()

        # ---------------- Phase B: causal attention -------------------
        # Heads processed in pairs with interleaved k-chunks: one head's
        # exp/mask/acc chain hides under the other's score/AV matmuls.

        def head_ctx(h):
            g = h // (HL // KVL)
            probsT, acc = get_head_tiles(h)
            pav = ps_av.tile([P, S], F32, tag="pav")
            return (h, g, probsT, acc, pav)

        def pieces_ki(ctx, ki):
            h, g, probsT, acc, pav = ctx
            q0 = ki * P
            kslice = slice(q0, q0 + P)
            for (a, b) in _pieces(q0, S):
                if (h, ki, a) in early_done:
                    continue
                psc = ps_small.tile([P, 512], F32, tag="ps")
                nc.tensor.matmul(psc[:, 0:b - a],
                                 sb_kT[:, g, kslice],
                                 sb_qT[:, h, a:b],
                                 start=True, stop=True)
                nc.scalar.activation(pt(probsT, ki, a, b),
                                     psc[:, 0:b - a], Exp,
                                     scale=exp_scale)
            # mask strictly-below-diagonal of the diag block on GpSimd
            nc.gpsimd.tensor_mul(pt(probsT, ki, q0, q0 + P),
                                 pt(probsT, ki, q0, q0 + P),
                                 utmask[:, :])
            # accumulate the column sums on DVE (2x bf16)
            if ki == 0:
                nc.vector.tensor_copy(acc[:, :], pt(probsT, 0, 0, S))
            else:
                nc.vector.tensor_add(acc[:, q0:], acc[:, q0:],
                                     pt(probsT, ki, q0, S))

        def av_ki(ctx, ki):
            h, g, probsT, acc, pav = ctx
            st, sp = (ki == 0), (ki == SC - 1)
            for (a, b) in _pieces(ki * P, S):
                nc.tensor.matmul(pav[:, a:b],
                                 sb_v[:, ki, g * D:(g + 1) * D],
                                 pt(probsT, ki, a, b),
                                 start=st, stop=sp)

        def finalize(ctx):
            h, g, probsT, acc, pav = ctx
            av_ki(ctx, SC - 1)
            # ones-matrix matmul = column sums broadcast across partitions
            rbc = battn.tile([P, S], F32, tag="rbc", bufs=1)
            for (a, b) in _pieces(0, S):
                psbc = ps_small.tile([P, 512], F32, tag="ps")
                nc.tensor.matmul(psbc[:, 0:b - a], ones_mat[:, :],
                                 acc[:, a:b], start=True, stop=True)
                nc.vector.reciprocal_approx_fast(rbc[:, a:b],
                                                 psbc[:, 0:b - a])
            nc.vector.tensor_mul(sb_attnT[:, h, :], pav[:, :], rbc[:, :])

        pending = [None]
        for hp in range(HL // 2):
            if pending[0] is not None:
                # finalize the previous pair before its pav slots rotate
                pending[0]()
                pending[0] = None
            ctxA = head_ctx(2 * hp)
            ctxB = head_ctx(2 * hp + 1)
            for ki in range(SC):
                pieces_ki(ctxA, ki)
                pieces_ki(ctxB, ki)
                if ki >= 1:
                    av_ki(ctxA, ki - 1)
                    av_ki(ctxB, ki - 1)

            def make_pending(cA, cB):
                def run():
                    finalize(cA)
                    finalize(cB)
                return run
            pending[0] = make_pending(ctxA, ctxB)

        # ---------------- Phase C: out projection ---------------------
        # Fine-grained: one 512-col PSUM slot per n-block with k-inner
        # accumulation.  m=0's first blocks run k<6 while the last pair's
        # finalize chains (attnT[6], attnT[7]) drain.
        ypool = tc.alloc_tile_pool(name="ysb", bufs=2)
        # the last pair finalizes first (before any phase-C ps_small allocs
        # so its ones-tiles don't rotate onto a held out_proj slot); m=0's
        # first blocks then run k<6 while attnT[6]/attnT[7] drain
        pending[0]()
        for m in range(SC):
            ms = slice(m * P, (m + 1) * P)
            last_m = (m == SC - 1)
            ysb = ypool.tile([P, HID], BF, tag="ysb")
            pys = {}
            for nb in range(HID // 512):
                nsl = slice(nb * 512, (nb + 1) * 512)
                py = ps_small.tile([P, 512], F32, tag="ps")
                if m == 0 and nb < 2:
                    pys[nb] = py
                    for k in range(HL - 2):
                        nc.tensor.matmul(py[:, :], sb_attnT[:, k, ms],
                                         sb_wo[:, k, nsl],
                                         start=(k == 0), stop=False)
                    if nb == 0:
                        continue
                    for pnb in (0, 1):
                        pnsl = slice(pnb * 512, (pnb + 1) * 512)
                        for k in (HL - 2, HL - 1):
                            nc.tensor.matmul(pys[pnb][:, :],
                                             sb_attnT[:, k, ms],
                                             sb_wo[:, k, pnsl],
                                             start=False, stop=(k == HL - 1))
                    nc.scalar.copy(ysb[:, 0:512], pys[0][:, :])
                else:
                    for k in range(HL):
                        nc.tensor.matmul(py[:, :],
                                         sb_attnT[:, k, ms],
                                         sb_wo[:, k, nsl],
                                         start=(k == 0), stop=(k == HL - 1))
                # both copy engines are idle in phase C: alternate
                if nb % 2 == 0:
                    nc.scalar.copy(ysb[:, nsl], py[:, :])
                    if last_m and nb == 2:
                        # tail: split the final stores across partition
                        # groups so each rides its own queue
                        for pg in range(4):
                            rs = slice(m * P + pg * 32, m * P + (pg + 1) * 32)
                            nc.sync.dma_start(out=out[rs, 1024:1536],
                                              in_=ysb[pg * 32:(pg + 1) * 32,
                                                      1024:1536])
                else:
                    nc.vector.tensor_copy(ysb[:, nsl], py[:, :])
                    if not last_m:
                        # store per 1024-col pair
                        nc.sync.dma_start(
                            out=out[ms, nb * 512 - 512:nb * 512 + 512],
                            in_=ysb[:, nb * 512 - 512:nb * 512 + 512])
                    elif nb == 1:
                        for pg in range(2):
                            rs = slice(m * P + pg * 64, m * P + (pg + 1) * 64)
                            nc.sync.dma_start(out=out[rs, 0:1024],
                                              in_=ysb[pg * 64:(pg + 1) * 64,
                                                      0:1024])
                    else:
                        for pg in range(4):
                            rs = slice(m * P + pg * 32, m * P + (pg + 1) * 32)
                            nc.sync.dma_start(out=out[rs, 1536:2048],
                                              in_=ysb[pg * 32:(pg + 1) * 32,
                                                      1536:2048])

        ypool.release()
        ps_av.release()
        ps_small.release()
        battn.release()
        qkvpool.release()
        wpool.release()
        cpool.release()

    nc.compile()
    return nc


def _get_nc():
    if "nc" not in _NC_CACHE:
        _NC_CACHE["nc"] = build_nc()
    return _NC_CACHE["nc"]


def _chunk_major(a, nchunks):
    """[nchunks*128, cols] -> [128, nchunks*cols] partition-major layout."""
    n = a.shape[1]
    return np.ascontiguousarray(
        a.reshape(nchunks, P, n).transpose(1, 0, 2).reshape(P, nchunks * n))


def _make_in_maps(x, cos, sin, wq, wk, wv, wo):
    bf = ml_dtypes.bfloat16
    HALF = D // 2
    sin_rot = np.concatenate([-sin[:, :HALF], sin[:, HALF:]], axis=1)
    cos_t = _chunk_major(cos, SC).astype(bf)
    sin_t = _chunk_major(sin_rot, SC).astype(bf)
    in_maps = []
    for core in range(NCORES):
        b, t = divmod(core, TP)
        wqkv = np.concatenate([
            wq[:, t * QD:(t + 1) * QD],
            wk[:, t * KD:(t + 1) * KD],
            wv[:, t * KD:(t + 1) * KD],
        ], axis=1)
        in_maps.append({
            "xT": _chunk_major(np.ascontiguousarray(x[b].T), KC).astype(bf),
            "wqkv": _chunk_major(wqkv, KC).astype(bf),
            "wo": _chunk_major(wo[t * QD:(t + 1) * QD, :], HL).astype(bf),
            "cos_t": cos_t, "sin_t": sin_t,
        })
    return in_maps


def run(inputs, trace=False):
    if trace:
        _ensure_ntff_hook()
    nc = _get_nc()
    in_maps = _make_in_maps(
        np.asarray(inputs["x"], np.float32),
        np.asarray(inputs["cos"], np.float32),
        np.asarray(inputs["sin"], np.float32),
        np.asarray(inputs["wq"], np.float32),
        np.asarray(inputs["wk"], np.float32),
        np.asarray(inputs["wv"], np.float32),
        np.asarray(inputs["wo"], np.float32),
    )
    try:
        res = run_bass_kernel_spmd(nc, in_maps, list(range(NCORES)),
                                   trace=trace)
    except Exception:
        # one retry: a previous process can leave a core wedged
        res = run_bass_kernel_spmd(nc, in_maps, list(range(NCORES)),
                                   trace=trace)
    outs = [np.asarray(r["out"]).astype(np.float32) for r in res.results]
    y = np.stack([outs[TP * b] + outs[TP * b + 1] for b in range(B)])
    return y, res


def kernel(**inputs):
    y, _ = run(inputs, trace=False)
    return y


# revision 13
# speedup vs baseline: 1.0669x; 1.0669x over previous
"""GQA causal attention with RoPE, distributed over 8 trn2 NeuronCores.

Sharding: 4-way data parallel over batch x 2-way tensor parallel over heads.
Core c = 2*b + t handles batch b with query heads [t*8, (t+1)*8) and KV heads
[t*2, (t+1)*2).  Each core computes a row-sharded out_proj partial; the pair
partials are summed on the host during unsharding.

On-chip algorithm (per core, bf16 matmuls / fp32 softmax):
  1. QKV projections from host-prearranged partition-major inputs
     (xT/wqkv/wo stored as [128, chunks*cols] so every DMA descriptor is a
     full 2-chunk 4-6KB partition line; the first chunk is split across
     4 partition-group DMAs so the first matmul starts ~1.5us after the
     queues open).  Token chunks m=0,1 and m=2's q-columns are projected
     k-outer while the input streams; the PSUM pools are split
     (q [P,1024] bufs=3 / kv [P,512] bufs=2) so three chunks are in
     flight during the DMA ramp.
  2. Each projection chunk is staged PSUM->SBUF (bf16) on ScalarE; RoPE
     runs all-bf16 on the DVE (2x perf mode).  The q transposes are issued
     on the ScalarE HWDGE ring and the k transposes on the sync ring so
     they never FIFO behind the bulk input stream, and the last chunk
     projects its q columns first so phase B's qT dependency lands before
     the early score pieces run out.
  3. Scores computed TRANSPOSED (scoresT[k_tok, q_tok]) so no probs
     transpose is needed: exp on ScalarE, column sums accumulated on DVE,
     summed across partitions by a ones-matmul, AV matmul consumes probsT
     directly, normalization happens once on the attention output.
  4. Causality: blocks with ki > qi are never computed; the diagonal block
     is masked with a precomputed upper-triangular 0/1 mask after exp.
     Heads are processed in PAIRS with interleaved k-chunks so the
     score->exp->AV chain of one head hides under the other's matmuls
     (phase B paces on ScalarE exp throughput).
  5. out_proj from feature-major attnT with wo as the moving operand,
     fine-grained (one 512-col PSUM bank per block, k-inner accumulation).
     The last chunk's output stores are split across partition groups /
     queues so the tail after the final matmul is ~1.5us.
"""

import math
import sys

sys.path.insert(0, "/opt/trn_rl_repo")

import ml_dtypes
import numpy as np

import concourse.bacc as bacc
import concourse.mybir as mybir
import concourse.tile as tile
from concourse.bass import _add_dep_helper
from concourse.bass_utils import run_bass_kernel_spmd
from concourse.masks import make_upper_triangular

B, S, HID = 4, 1024, 2048
H, KV, D = 16, 4, 128
P = 128
TP = 2                  # tensor-parallel ways (head split)
HL = H // TP            # 8 query heads per core
KVL = KV // TP          # 2 kv heads per core
QD = HL * D             # 1024
KD = KVL * D            # 256
SC = S // P             # 8 token chunks
KC = HID // P           # 16 hidden chunks
QKVD = QD + 2 * KD      # 1536 = q 1024 | k 256 | v 256
NCORES = 8
BF = mybir.dt.bfloat16
F32 = mybir.dt.float32
Exp = mybir.ActivationFunctionType.Exp

_NC_CACHE = {}


def _ensure_ntff_hook():
    """The agent image's antenv lacks axon_hooks, so bass_utils' trace=True
    path can't find the NTFF profile hook trn_boot would have registered.
    Recreate the module and register the ctypes-based hook ourselves."""
    try:
        from antenv.axon_hooks import get_axon_ntff_profile_hook  # noqa: F401
        return
    except ImportError:
        pass
    import types

    import antenv

    mod = types.ModuleType("antenv.axon_hooks")
    _state = {"hook": None}
    mod.set_axon_ntff_profile_hook = lambda h: _state.__setitem__("hook", h)
    mod.get_axon_ntff_profile_hook = lambda: _state["hook"]
    sys.modules["antenv.axon_hooks"] = mod
    antenv.axon_hooks = mod
    try:
        from trn_agent_boot.trn_boot import _ntff_profile_via_ctypes

        hook = _ntff_profile_via_ctypes("/opt/axon/libaxon_pjrt.so")
        if hook is not None:
            mod.set_axon_ntff_profile_hook(hook)
    except Exception as e:  # pragma: no cover
        print(f"NTFF hook registration failed: {e}", file=sys.stderr)


def _pieces(start, end, step=512):
    """Split [start, end) into spans of at most `step`, aligned so no span
    crosses a `step` boundary (PSUM: one bank per matmul)."""
    out = []
    a = start
    while a < end:
        b = min((a // step + 1) * step, end)
        out.append((a, b))
        a = b
    return out


def build_nc():
    nc = bacc.Bacc("TRN2", target_bir_lowering=False, debug=False,
                   num_devices=NCORES)

    # partition-major DRAM layouts: row p holds chunk-c data at [c*cols ...]
    xT = nc.declare_dram_parameter("xT", [P, KC * S], BF, isOutput=False)
    wqkv = nc.declare_dram_parameter("wqkv", [P, KC * QKVD], BF, isOutput=False)
    wo = nc.declare_dram_parameter("wo", [P, HL * HID], BF, isOutput=False)
    cos_t = nc.declare_dram_parameter("cos_t", [P, SC * D], BF, isOutput=False)
    sin_t = nc.declare_dram_parameter("sin_t", [P, SC * D], BF, isOutput=False)
    out = nc.declare_dram_parameter("out", [S, HID], BF, isOutput=True)

    with tile.TileContext(nc) as tc:
        # ---- persistent pools (allocated first: fixed addresses) ----
        cpool = tc.alloc_tile_pool(name="consts", bufs=1)
        wpool = tc.alloc_tile_pool(name="wpool", bufs=1)
        qkvpool = tc.alloc_tile_pool(name="qkvpool", bufs=1)
        # phase B's SBUF pool allocated BEFORE phase A's pools so their
        # address ranges are disjoint: no release barrier between A and B.
        battn = tc.alloc_tile_pool(name="battn", bufs=2)

        utmask = cpool.tile([P, P], BF)
        ones_mat = cpool.tile([P, P], BF)

        sb_wo = wpool.tile([P, HL, HID], BF)

        # combined feature-major q+k so each chunk's RoPE output transposes
        # with a single DMA kick: groups 0..7 = q heads, 8..9 = k heads
        sb_qkT = qkvpool.tile([P, HL + KVL, S], BF)
        sb_qT = sb_qkT[:, 0:HL, :]
        sb_kT = sb_qkT[:, HL:HL + KVL, :]
        sb_v = qkvpool.tile([P, SC, KD], BF)      # token-major v
        sb_attnT = qkvpool.tile([P, HL, S], BF)   # feature-major attn out

        # ---------------- Phase A: projections + RoPE -----------------
        projpool = tc.alloc_tile_pool(name="proj", bufs=1)
        ropepool = tc.alloc_tile_pool(name="rope", bufs=2)
        # split projection PSUM: q-part [P,1024] (2 banks) x3, kv-part
        # [P,512] (1 bank) x2 -> 8 banks, three chunks in flight at ramp
        ps_qq = tc.alloc_tile_pool(name="ps_qq", bufs=3, space="PSUM")
        ps_kv = tc.alloc_tile_pool(name="ps_kv", bufs=2, space="PSUM")

        sb_xT = projpool.tile([P, KC, S], BF)
        sb_wqkv = projpool.tile([P, KC, QKVD], BF)

        # every dma_start costs ~1.2us of serialized HWDGE-ring time (the
        # kick + descriptor generation) while the descriptors themselves
        # stream in parallel across 16 queues -- so the input stream is
        # organized to MINIMIZE dma_start count: a 4-kick fast path for the
        # first matmuls, then 4-chunk quads, and a single-kick wo load.
        nc.sync.dma_start(out=sb_wqkv[:, 0, 0:512], in_=wqkv[:, 0:512])
        nc.sync.dma_start(out=sb_xT[:, 0, 0:384], in_=xT[:, 0:384])
        nc.sync.dma_start(out=sb_wqkv[:, 0, 512:QKVD], in_=wqkv[:, 512:QKVD])
        nc.sync.dma_start(out=sb_xT[:, 0, 384:S], in_=xT[:, 384:S])
        nc.sync.dma_start(out=sb_wqkv[:, 1, :],
                          in_=wqkv[:, QKVD:2 * QKVD])
        nc.sync.dma_start(out=sb_xT[:, 1, :], in_=xT[:, S:2 * S])
        sb_ck = projpool.tile([P, SC, D], BF)
        nc.sync.dma_start(out=sb_ck[:, :, :], in_=cos_t[:, :])
        sb_sk = projpool.tile([P, SC, D], BF)
        nc.sync.dma_start(out=sb_sk[:, :, :], in_=sin_t[:, :])
        for c in range(2, KC, 4):
            cn = min(c + 4, KC)
            nc.sync.dma_start(out=sb_wqkv[:, c:cn, :],
                              in_=wqkv[:, c * QKVD:cn * QKVD])
            nc.sync.dma_start(out=sb_xT[:, c:cn, :],
                              in_=xT[:, c * S:cn * S])
        # wo is only needed in phase C: delay its (single-kick, 16KB/line)
        # load until the input streaming has drained (dep added below)
        wo_dma = nc.sync.dma_start(out=sb_wo[:, :, :], in_=wo[:, :])

        # mask/ones builders issued after the DMA starts so the sync engine
        # kicks off the input stream first (they are not needed until B)
        make_upper_triangular(nc, utmask[:, :], val=1.0, diag=True)
        nc.vector.memset(ones_mat[:, :], 1.0)

        HALF = D // 2

        def rope_block(sb_src, lo, nh, m):
            """RoPE `nh` consecutive heads of the staged bf16 chunk (cols
            [lo, lo+nh*D)) in one batched op per step, via free-dim-broadcast
            cos/sin APs.  All-bf16 so the DVE runs in 2x mode.  Returns a
            bf16 SBUF tile [P, nh*D]."""
            # t1 is produced+consumed inside one in-order DVE chain: bufs=1
            t1 = ropepool.tile([P, nh, D], BF, tag="t1", bufs=1)
            ro = ropepool.tile([P, nh * D], BF, tag="ro", bufs=2)
            src = sb_src[:, lo:lo + nh * D].rearrange("p (h d) -> p h d", h=nh)
            sin_lo = sb_sk[:, m:m + 1, 0:HALF].broadcast_to([P, nh, HALF])
            sin_hi = sb_sk[:, m:m + 1, HALF:D].broadcast_to([P, nh, HALF])
            cos_b = sb_ck[:, m:m + 1, :].broadcast_to([P, nh, D])
            # rot_half * sin (sin table pre-negated on first half)
            nc.vector.tensor_mul(t1[:, :, 0:HALF], src[:, :, HALF:D], sin_lo)
            nc.vector.tensor_mul(t1[:, :, HALF:D], src[:, :, 0:HALF], sin_hi)
            ror = ro[:, :].rearrange("p (h d) -> p h d", h=nh)
            # ro = src*cos + t1
            nc.vector.tensor_mul(ror, src, cos_b)
            nc.vector.tensor_add(ror, ror, t1[:, :, :])
            return ro

        def proj_q(pq, m, k):
            st, sp = (k == 0), (k == KC - 1)
            lhsT = sb_xT[:, k, m * P:(m + 1) * P]
            for n in (0, 1):
                mm = nc.tensor.matmul(
                    pq[:, n * 512:(n + 1) * 512], lhsT,
                    sb_wqkv[:, k, n * 512:(n + 1) * 512],
                    start=st, stop=sp)
            return mm

        def proj_kv(pkv, m, k):
            st, sp = (k == 0), (k == KC - 1)
            lhsT = sb_xT[:, k, m * P:(m + 1) * P]
            return nc.tensor.matmul(
                pkv[:, :], lhsT, sb_wqkv[:, k, 1024:QKVD],
                start=st, stop=sp)

        def stage_q(pq):
            sb_qk = ropepool.tile([P, QD + KD], BF, tag="qk")
            nc.scalar.copy(sb_qk[:, 0:512], pq[:, 0:512])
            nc.scalar.copy(sb_qk[:, 512:QD], pq[:, 512:QD])
            return sb_qk

        def stage_kv(sb_qk, pkv, m):
            nc.scalar.copy(sb_qk[:, QD:QD + KD], pkv[:, 0:KD])
            nc.scalar.copy(sb_v[:, m, :], pkv[:, KD:2 * KD])

        def finish_m(pq, pkv, m):
            # combined q+k RoPE (one 4-op DVE chain over 10 head-groups)
            # and ONE transpose kick per chunk, on the ScalarE (Act) HWDGE
            # ring so it never FIFOs behind the bulk input stream
            sb_qk = stage_q(pq)
            stage_kv(sb_qk, pkv, m)
            ms = slice(m * P, (m + 1) * P)
            qk_ro = rope_block(sb_qk, 0, HL + KVL, m)
            nc.scalar.dma_start_transpose(out=sb_qkT[:, :, ms], in_=qk_ro[:, :])

        # m=0, m=1 and m=2's q columns share each arriving k-chunk during
        # the DMA ramp (three chunks in flight across the split PSUM pools)
        pq0 = ps_qq.tile([P, QD], F32, tag="pq")
        pkv0 = ps_kv.tile([P, 2 * KD], F32, tag="pkv")
        pq1 = ps_qq.tile([P, QD], F32, tag="pq")
        pkv1 = ps_kv.tile([P, 2 * KD], F32, tag="pkv")
        pq2 = ps_qq.tile([P, QD], F32, tag="pq")
        for k in range(KC):
            proj_q(pq0, 0, k)
            proj_kv(pkv0, 0, k)
            proj_q(pq1, 1, k)
            proj_kv(pkv1, 1, k)
            mm = proj_q(pq2, 2, k)
        # release the wo load only once the input streaming has drained
        _add_dep_helper(wo_dma.ins, mm.ins,
                        reason="delay wo load past input ramp")
        finish_m(pq0, pkv0, 0)
        pkv2 = ps_kv.tile([P, 2 * KD], F32, tag="pkv")
        for k in range(KC):
            proj_kv(pkv2, 2, k)
        finish_m(pq1, pkv1, 1)
        prev = (pq2, pkv2, 2)
        for m in range(3, SC - 1):
            pq = ps_qq.tile([P, QD], F32, tag="pq")
            pkv = ps_kv.tile([P, 2 * KD], F32, tag="pkv")
            # q columns for all k first, kv columns after: the kv tile's
            # slot WAR (stage_kv two chunks back) has drained by then
            for k in range(KC):
                proj_q(pq, m, k)
            for k in range(KC):
                proj_kv(pkv, m, k)
            finish_m(*prev)
            prev = (pq, pkv, m)

        # Last chunk: q columns first so its qT (needed by phase B almost
        # immediately) lands while the PE still has kv-proj + early score
        # work; k/v columns follow and kT rides the sync ring in parallel.
        M7 = SC - 1
        pq7 = ps_qq.tile([P, QD], F32, tag="pq")
        for k in range(KC):
            proj_q(pq7, M7, k)
        finish_m(*prev)
        sb_qk7 = stage_q(pq7)
        ms7 = slice(M7 * P, (M7 + 1) * P)
        q7_ro = rope_block(sb_qk7, 0, HL, M7)
        # both last-chunk transposes ride the (by now idle) sync ring so
        # the Scalar FIFO can run stage_kv(7) + the early exps immediately
        nc.sync.dma_start_transpose(out=sb_qT[:, :, ms7], in_=q7_ro[:, :])
        pkv7 = ps_kv.tile([P, 2 * KD], F32, tag="pkv")
        for k in range(KC):
            proj_kv(pkv7, M7, k)
        stage_kv(sb_qk7, pkv7, M7)
        ps_kv.release()
        ps_qq.release()
        # Shared 4-slot [P,512] fp32 PSUM pool for score tiles, ones-matmul
        # sum tiles AND phase C's out_proj tiles; 2-slot [P,S] pool for the
        # paired heads' AV accumulators.
        ps_small = tc.alloc_tile_pool(name="ps_small", bufs=4, space="PSUM")
        ps_av = tc.alloc_tile_pool(name="ps_av", bufs=2, space="PSUM")

        exp_scale = float(1 / math.sqrt(D))
        # ragged probsT: row ki only stores the causal columns [ki*P, S)
        OFF = [0]
        for ki in range(SC):
            OFF.append(OFF[-1] + S - ki * P)
        PTOT = OFF[-1]          # 4608

        head_tiles = {}

        def get_head_tiles(h):
            if h not in head_tiles:
                probsT = battn.tile([P, PTOT], BF, tag="probsT", bufs=3,
                                    name=f"probsT{h}")
                acc = battn.tile([P, S], BF, tag="acc", bufs=3,
                                 name=f"acc{h}")
                head_tiles[h] = (probsT, acc)
            return head_tiles[h]

        def pt(probsT, ki, a, b):
            q0 = ki * P
            return probsT[:, OFF[ki] + (a - q0):OFF[ki] + (b - q0)]

        # early score pieces: cover the PE while the last chunk's RoPE +
        # transposes drain (they only touch token chunks 0-3)
        early_done = set()
        for h in (0, 1, 2):
            probsT, _ = get_head_tiles(h)
            for ki in range(4):
                a = ki * P
                psc = ps_small.tile([P, 512], F32, tag="ps", name="psce")
                nc.tensor.matmul(psc[:, 0:512 - a], sb_kT[:, 0, a:a + P],
                                 sb_qT[:, h, a:512], start=True, stop=True)
                nc.scalar.activation(pt(probsT, ki, a, 512),
                                     psc[:, 0:512 - a],
                                     Exp, scale=exp_scale)
                early_done.add((h, ki, a))

        k7_ro = rope_block(sb_qk7, QD, KVL, M7)
        nc.sync.dma_start_transpose(out=sb_kT[:, :, ms7], in_=k7_ro[:, :])
        ropepool.release()
        projpool.release()

        # ---------------- Phase B: causal attention -------------------
        # Heads processed in pairs with interleaved k-chunks: one head's
        # exp/mask/acc chain hides under the other's score/AV matmuls.

        def head_ctx(h):
            g = h // (HL // KVL)
            probsT, acc = get_head_tiles(h)
            pav = ps_av.tile([P, S], F32, tag="pav")
            return (h, g, probsT, acc, pav)

        def pieces_ki(ctx, ki):
            h, g, probsT, acc, pav = ctx
            q0 = ki * P
            kslice = slice(q0, q0 + P)
            for (a, b) in _pieces(q0, S):
                if (h, ki, a) in early_done:
                    continue
                psc = ps_small.tile([P, 512], F32, tag="ps")
                nc.tensor.matmul(psc[:, 0:b - a],
                                 sb_kT[:, g, kslice],
                                 sb_qT[:, h, a:b],
                                 start=True, stop=True)
                nc.scalar.activation(pt(probsT, ki, a, b),
                                     psc[:, 0:b - a], Exp,
                                     scale=exp_scale)
            # mask strictly-below-diagonal of the diag block on GpSimd
            nc.gpsimd.tensor_mul(pt(probsT, ki, q0, q0 + P),
                                 pt(probsT, ki, q0, q0 + P),
                                 utmask[:, :])
            # accumulate the column sums on DVE (2x bf16)
            if ki == 0:
                nc.vector.tensor_copy(acc[:, :], pt(probsT, 0, 0, S))
            else:
                nc.vector.tensor_add(acc[:, q0:], acc[:, q0:],
                                     pt(probsT, ki, q0, S))

        def av_ki(ctx, ki):
            h, g, probsT, acc, pav = ctx
            st, sp = (ki == 0), (ki == SC - 1)
            for (a, b) in _pieces(ki * P, S):
                nc.tensor.matmul(pav[:, a:b],
                                 sb_v[:, ki, g * D:(g + 1) * D],
                                 pt(probsT, ki, a, b),
                                 start=st, stop=sp)

        def finalize(ctx):
            h, g, probsT, acc, pav = ctx
            av_ki(ctx, SC - 1)
            # ones-matrix matmul = column sums broadcast across partitions
            rbc = battn.tile([P, S], F32, tag="rbc", bufs=1)
            for (a, b) in _pieces(0, S):
                psbc = ps_small.tile([P, 512], F32, tag="ps")
                nc.tensor.matmul(psbc[:, 0:b - a], ones_mat[:, :],
                                 acc[:, a:b], start=True, stop=True)
                nc.vector.reciprocal_approx_fast(rbc[:, a:b],
                                                 psbc[:, 0:b - a])
            nc.vector.tensor_mul(sb_attnT[:, h, :], pav[:, :], rbc[:, :])

        pending = [None]
        for hp in range(HL // 2):
            if pending[0] is not None:
                # finalize the previous pair before its pav slots rotate
                pending[0]()
                pending[0] = None
            ctxA = head_ctx(2 * hp)
            ctxB = head_ctx(2 * hp + 1)
            for ki in range(SC):
                pieces_ki(ctxA, ki)
                pieces_ki(ctxB, ki)
                if ki >= 1:
                    av_ki(ctxA, ki - 1)
                    av_ki(ctxB, ki - 1)

            def make_pending(cA, cB):
                def run():
                    finalize(cA)
                    finalize(cB)
                return run
            pending[0] = make_pending(ctxA, ctxB)

        # ---------------- Phase C: out projection ---------------------
        # Fine-grained: one 512-col PSUM slot per n-block with k-inner
        # accumulation.  m=0's first blocks run k<6 while the last pair's
        # finalize chains (attnT[6], attnT[7]) drain.
        ypool = tc.alloc_tile_pool(name="ysb", bufs=2)
        # the last pair finalizes first (before any phase-C ps_small allocs
        # so its ones-tiles don't rotate onto a held out_proj slot); m=0's
        # first blocks then run k<6 while attnT[6]/attnT[7] drain
        pending[0]()
        for m in range(SC):
            ms = slice(m * P, (m + 1) * P)
            last_m = (m == SC - 1)
            ysb = ypool.tile([P, HID], BF, tag="ysb")
            pys = {}
            for nb in range(HID // 512):
                nsl = slice(nb * 512, (nb + 1) * 512)
                py = ps_small.tile([P, 512], F32, tag="ps")
                if m == 0 and nb < 2:
                    pys[nb] = py
                    for k in range(HL - 2):
                        nc.tensor.matmul(py[:, :], sb_attnT[:, k, ms],
                                         sb_wo[:, k, nsl],
                                         start=(k == 0), stop=False)
                    if nb == 0:
                        continue
                    for pnb in (0, 1):
                        pnsl = slice(pnb * 512, (pnb + 1) * 512)
                        for k in (HL - 2, HL - 1):
                            nc.tensor.matmul(pys[pnb][:, :],
                                             sb_attnT[:, k, ms],
                                             sb_wo[:, k, pnsl],
                                             start=False, stop=(k == HL - 1))
                    nc.scalar.copy(ysb[:, 0:512], pys[0][:, :])
                else:
                    for k in range(HL):
                        nc.tensor.matmul(py[:, :],
                                         sb_attnT[:, k, ms],
                                         sb_wo[:, k, nsl],
                                         start=(k == 0), stop=(k == HL - 1))
                # both copy engines are idle in phase C: alternate
                if nb % 2 == 0:
                    nc.scalar.copy(ysb[:, nsl], py[:, :])
                    if last_m and nb == 2:
                        # tail: store this block early so the final kick
                        # only moves the last 128KB
                        nc.sync.dma_start(out=out[ms, 1024:1536],
                                          in_=ysb[:, 1024:1536])
                else:
                    nc.vector.tensor_copy(ysb[:, nsl], py[:, :])
                    if not last_m:
                        # store per 1024-col pair
                        nc.sync.dma_start(
                            out=out[ms, nb * 512 - 512:nb * 512 + 512],
                            in_=ysb[:, nb * 512 - 512:nb * 512 + 512])
                    elif nb == 1:
                        nc.sync.dma_start(out=out[ms, 0:1024],
                                          in_=ysb[:, 0:1024])
                    else:
                        nc.sync.dma_start(out=out[ms, 1536:2048],
                                          in_=ysb[:, 1536:2048])

        ypool.release()
        ps_av.release()
        ps_small.release()
        battn.release()
        qkvpool.release()
        wpool.release()
        cpool.release()

    nc.compile()
    return nc


def _get_nc():
    if "nc" not in _NC_CACHE:
        _NC_CACHE["nc"] = build_nc()
    return _NC_CACHE["nc"]


def _chunk_major(a, nchunks):
    """[nchunks*128, cols] -> [128, nchunks*cols] partition-major layout."""
    n = a.shape[1]
    return np.ascontiguousarray(
        a.reshape(nchunks, P, n).transpose(1, 0, 2).reshape(P, nchunks * n))


def _make_in_maps(x, cos, sin, wq, wk, wv, wo):
    bf = ml_dtypes.bfloat16
    HALF = D // 2
    sin_rot = np.concatenate([-sin[:, :HALF], sin[:, HALF:]], axis=1)
    cos_t = _chunk_major(cos, SC).astype(bf)
    sin_t = _chunk_major(sin_rot, SC).astype(bf)
    in_maps = []
    for core in range(NCORES):
        b, t = divmod(core, TP)
        wqkv = np.concatenate([
            wq[:, t * QD:(t + 1) * QD],
            wk[:, t * KD:(t + 1) * KD],
            wv[:, t * KD:(t + 1) * KD],
        ], axis=1)
        in_maps.append({
            "xT": _chunk_major(np.ascontiguousarray(x[b].T), KC).astype(bf),
            "wqkv": _chunk_major(wqkv, KC).astype(bf),
            "wo": _chunk_major(wo[t * QD:(t + 1) * QD, :], HL).astype(bf),
            "cos_t": cos_t, "sin_t": sin_t,
        })
    return in_maps


def run(inputs, trace=False):
    if trace:
        _ensure_ntff_hook()
    nc = _get_nc()
    in_maps = _make_in_maps(
        np.asarray(inputs["x"], np.float32),
        np.asarray(inputs["cos"], np.float32),
        np.asarray(inputs["sin"], np.float32),
        np.asarray(inputs["wq"], np.float32),
        np.asarray(inputs["wk"], np.float32),
        np.asarray(inputs["wv"], np.float32),
        np.asarray(inputs["wo"], np.float32),
    )
    try:
        res = run_bass_kernel_spmd(nc, in_maps, list(range(NCORES)),
                                   trace=trace)
    except Exception:
        # one retry: a previous process can leave a core wedged
        res = run_bass_kernel_spmd(nc, in_maps, list(range(NCORES)),
                                   trace=trace)
    outs = [np.asarray(r["out"]).astype(np.float32) for r in res.results]
    y = np.stack([outs[TP * b] + outs[TP * b + 1] for b in range(B)])
    return y, res


def kernel(**inputs):
    y, _ = run(inputs, trace=False)
    return y


# revision 18
# speedup vs baseline: 1.1103x; 1.0406x over previous
"""GQA causal attention with RoPE, distributed over 8 trn2 NeuronCores.

Sharding: 4-way data parallel over batch x 2-way tensor parallel over heads.
Core c = 2*b + t handles batch b with query heads [t*8, (t+1)*8) and KV heads
[t*2, (t+1)*2).  Each core computes a row-sharded out_proj partial; the pair
partials are summed on the host during unsharding.

On-chip algorithm (per core, bf16 matmuls / fp32 softmax):
  1. QKV projections from host-prearranged partition-major inputs
     (xT/wqkv/wo stored as [128, chunks*cols] so every DMA descriptor is a
     full 2-chunk 4-6KB partition line; the first chunk is split across
     4 partition-group DMAs so the first matmul starts ~1.5us after the
     queues open).  Token chunks m=0,1 and m=2's q-columns are projected
     k-outer while the input streams; the PSUM pools are split
     (q [P,1024] bufs=3 / kv [P,512] bufs=2) so three chunks are in
     flight during the DMA ramp.
  2. Each projection chunk is staged PSUM->SBUF (bf16) on ScalarE; RoPE
     runs all-bf16 on the DVE (2x perf mode).  The q transposes are issued
     on the ScalarE HWDGE ring and the k transposes on the sync ring so
     they never FIFO behind the bulk input stream, and the last chunk
     projects its q columns first so phase B's qT dependency lands before
     the early score pieces run out.
  3. Scores computed TRANSPOSED (scoresT[k_tok, q_tok]) so no probs
     transpose is needed: exp on ScalarE, column sums accumulated on DVE,
     summed across partitions by a ones-matmul, AV matmul consumes probsT
     directly, normalization happens once on the attention output.
  4. Causality: blocks with ki > qi are never computed; the diagonal block
     is masked with a precomputed upper-triangular 0/1 mask after exp.
     Heads are processed in PAIRS with interleaved k-chunks so the
     score->exp->AV chain of one head hides under the other's matmuls
     (phase B paces on ScalarE exp throughput).
  5. out_proj from feature-major attnT with wo as the moving operand,
     fine-grained (one 512-col PSUM bank per block, k-inner accumulation).
     The last chunk's output stores are split across partition groups /
     queues so the tail after the final matmul is ~1.5us.
"""

import math
import sys

sys.path.insert(0, "/opt/trn_rl_repo")

import ml_dtypes
import numpy as np

import concourse.bacc as bacc
import concourse.mybir as mybir
import concourse.tile as tile
from concourse.bass import _add_dep_helper
from concourse.bass_utils import run_bass_kernel_spmd
from concourse.masks import make_upper_triangular

B, S, HID = 4, 1024, 2048
H, KV, D = 16, 4, 128
P = 128
TP = 2                  # tensor-parallel ways (head split)
HL = H // TP            # 8 query heads per core
KVL = KV // TP          # 2 kv heads per core
QD = HL * D             # 1024
KD = KVL * D            # 256
SC = S // P             # 8 token chunks
KC = HID // P           # 16 hidden chunks
QKVD = QD + 2 * KD      # 1536 = q 1024 | k 256 | v 256
NCORES = 8
BF = mybir.dt.bfloat16
F32 = mybir.dt.float32
Exp = mybir.ActivationFunctionType.Exp

_NC_CACHE = {}


def _ensure_ntff_hook():
    """The agent image's antenv lacks axon_hooks, so bass_utils' trace=True
    path can't find the NTFF profile hook trn_boot would have registered.
    Recreate the module and register the ctypes-based hook ourselves."""
    try:
        from antenv.axon_hooks import get_axon_ntff_profile_hook  # noqa: F401
        return
    except ImportError:
        pass
    import types

    import antenv

    mod = types.ModuleType("antenv.axon_hooks")
    _state = {"hook": None}
    mod.set_axon_ntff_profile_hook = lambda h: _state.__setitem__("hook", h)
    mod.get_axon_ntff_profile_hook = lambda: _state["hook"]
    sys.modules["antenv.axon_hooks"] = mod
    antenv.axon_hooks = mod
    try:
        from trn_agent_boot.trn_boot import _ntff_profile_via_ctypes

        hook = _ntff_profile_via_ctypes("/opt/axon/libaxon_pjrt.so")
        if hook is not None:
            mod.set_axon_ntff_profile_hook(hook)
    except Exception as e:  # pragma: no cover
        print(f"NTFF hook registration failed: {e}", file=sys.stderr)


def _pieces(start, end, step=512):
    """Split [start, end) into spans of at most `step`, aligned so no span
    crosses a `step` boundary (PSUM: one bank per matmul)."""
    out = []
    a = start
    while a < end:
        b = min((a // step + 1) * step, end)
        out.append((a, b))
        a = b
    return out


def build_nc():
    nc = bacc.Bacc("TRN2", target_bir_lowering=False, debug=False,
                   num_devices=NCORES)

    # partition-major DRAM layouts: row p holds chunk-c data at [c*cols ...]
    xT = nc.declare_dram_parameter("xT", [P, KC * S], BF, isOutput=False)
    wqkv = nc.declare_dram_parameter("wqkv", [P, KC * QKVD], BF, isOutput=False)
    wo = nc.declare_dram_parameter("wo", [P, HL * HID], BF, isOutput=False)
    cos_t = nc.declare_dram_parameter("cos_t", [P, SC * D], BF, isOutput=False)
    sin_t = nc.declare_dram_parameter("sin_t", [P, SC * D], BF, isOutput=False)
    out = nc.declare_dram_parameter("out", [S, HID], BF, isOutput=True)

    with tile.TileContext(nc) as tc:
        # ---- persistent pools (allocated first: fixed addresses) ----
        cpool = tc.alloc_tile_pool(name="consts", bufs=1)
        wpool = tc.alloc_tile_pool(name="wpool", bufs=1)
        qkvpool = tc.alloc_tile_pool(name="qkvpool", bufs=1)
        # phase B's SBUF pool allocated BEFORE phase A's pools so their
        # address ranges are disjoint: no release barrier between A and B.
        battn = tc.alloc_tile_pool(name="battn", bufs=2)

        utmask = cpool.tile([P, P], BF)
        ones_mat = cpool.tile([P, P], BF)

        sb_wo = wpool.tile([P, HL, HID], BF)

        # combined feature-major q+k so each chunk's RoPE output transposes
        # with a single DMA kick: groups 0..7 = q heads, 8..9 = k heads
        sb_qkT = qkvpool.tile([P, HL + KVL, S], BF)
        sb_qT = sb_qkT[:, 0:HL, :]
        sb_kT = sb_qkT[:, HL:HL + KVL, :]
        sb_v = qkvpool.tile([P, SC, KD], BF)      # token-major v
        sb_attnT = qkvpool.tile([P, HL, S], BF)   # feature-major attn out

        # ---------------- Phase A: projections + RoPE -----------------
        projpool = tc.alloc_tile_pool(name="proj", bufs=1)
        ropepool = tc.alloc_tile_pool(name="rope", bufs=2)
        # split projection PSUM: q-part [P,1024] (2 banks) x3, kv-part
        # [P,512] (1 bank) x2 -> 8 banks, three chunks in flight at ramp
        ps_qq = tc.alloc_tile_pool(name="ps_qq", bufs=3, space="PSUM")
        ps_kv = tc.alloc_tile_pool(name="ps_kv", bufs=2, space="PSUM")

        sb_xT = projpool.tile([P, KC, S], BF)
        sb_wqkv = projpool.tile([P, KC, QKVD], BF)

        # every dma_start costs ~1.2us of serialized HWDGE-ring time (the
        # kick + descriptor generation) while the descriptors themselves
        # stream in parallel across 16 queues -- so the input stream is
        # organized to MINIMIZE dma_start count: a 4-kick fast path for the
        # first matmuls, then 4-chunk quads, and a single-kick wo load.
        nc.sync.dma_start(out=sb_wqkv[:, 0, 0:512], in_=wqkv[:, 0:512])
        nc.sync.dma_start(out=sb_xT[:, 0, 0:384], in_=xT[:, 0:384])
        nc.sync.dma_start(out=sb_wqkv[:, 0, 512:QKVD], in_=wqkv[:, 512:QKVD])
        nc.sync.dma_start(out=sb_xT[:, 0, 384:S], in_=xT[:, 384:S])
        nc.sync.dma_start(out=sb_wqkv[:, 1, :],
                          in_=wqkv[:, QKVD:2 * QKVD])
        nc.sync.dma_start(out=sb_xT[:, 1, :], in_=xT[:, S:2 * S])
        sb_ck = projpool.tile([P, SC, D], BF)
        nc.sync.dma_start(out=sb_ck[:, :, :], in_=cos_t[:, :])
        sb_sk = projpool.tile([P, SC, D], BF)
        nc.sync.dma_start(out=sb_sk[:, :, :], in_=sin_t[:, :])
        # pairs while the PE ramp is hungry, quads once it has a backlog
        for c, cn in ((2, 4), (4, 6), (6, 10), (10, 14), (14, 16)):
            nc.sync.dma_start(out=sb_wqkv[:, c:cn, :],
                              in_=wqkv[:, c * QKVD:cn * QKVD])
            nc.sync.dma_start(out=sb_xT[:, c:cn, :],
                              in_=xT[:, c * S:cn * S])
        # wo is only needed in phase C: delay its (single-kick, 16KB/line)
        # load until the input streaming has drained (dep added below)
        wo_dma = nc.sync.dma_start(out=sb_wo[:, :, :], in_=wo[:, :])

        # mask/ones builders issued after the DMA starts so the sync engine
        # kicks off the input stream first (they are not needed until B)
        make_upper_triangular(nc, utmask[:, :], val=1.0, diag=True)
        nc.vector.memset(ones_mat[:, :], 1.0)

        HALF = D // 2

        def rope_block(sb_src, lo, nh, m):
            """RoPE `nh` consecutive heads of the staged bf16 chunk (cols
            [lo, lo+nh*D)) in one batched op per step, via free-dim-broadcast
            cos/sin APs.  All-bf16 so the DVE runs in 2x mode.  Returns a
            bf16 SBUF tile [P, nh*D]."""
            # t1 is produced+consumed inside one in-order DVE chain: bufs=1
            t1 = ropepool.tile([P, nh, D], BF, tag="t1", bufs=1)
            ro = ropepool.tile([P, nh * D], BF, tag="ro", bufs=2)
            src = sb_src[:, lo:lo + nh * D].rearrange("p (h d) -> p h d", h=nh)
            sin_lo = sb_sk[:, m:m + 1, 0:HALF].broadcast_to([P, nh, HALF])
            sin_hi = sb_sk[:, m:m + 1, HALF:D].broadcast_to([P, nh, HALF])
            cos_b = sb_ck[:, m:m + 1, :].broadcast_to([P, nh, D])
            # rot_half * sin (sin table pre-negated on first half)
            nc.vector.tensor_mul(t1[:, :, 0:HALF], src[:, :, HALF:D], sin_lo)
            nc.vector.tensor_mul(t1[:, :, HALF:D], src[:, :, 0:HALF], sin_hi)
            ror = ro[:, :].rearrange("p (h d) -> p h d", h=nh)
            # ro = src*cos + t1
            nc.vector.tensor_mul(ror, src, cos_b)
            nc.vector.tensor_add(ror, ror, t1[:, :, :])
            return ro

        def proj_q(pq, m, k):
            st, sp = (k == 0), (k == KC - 1)
            lhsT = sb_xT[:, k, m * P:(m + 1) * P]
            for n in (0, 1):
                mm = nc.tensor.matmul(
                    pq[:, n * 512:(n + 1) * 512], lhsT,
                    sb_wqkv[:, k, n * 512:(n + 1) * 512],
                    start=st, stop=sp)
            return mm

        def proj_kv(pkv, m, k):
            st, sp = (k == 0), (k == KC - 1)
            lhsT = sb_xT[:, k, m * P:(m + 1) * P]
            return nc.tensor.matmul(
                pkv[:, :], lhsT, sb_wqkv[:, k, 1024:QKVD],
                start=st, stop=sp)

        def stage_q(pq):
            sb_qk = ropepool.tile([P, QD + KD], BF, tag="qk")
            nc.scalar.copy(sb_qk[:, 0:512], pq[:, 0:512])
            nc.scalar.copy(sb_qk[:, 512:QD], pq[:, 512:QD])
            return sb_qk

        def stage_kv(sb_qk, pkv, m):
            nc.scalar.copy(sb_qk[:, QD:QD + KD], pkv[:, 0:KD])
            nc.scalar.copy(sb_v[:, m, :], pkv[:, KD:2 * KD])

        def finish_m(pq, pkv, m):
            # combined q+k RoPE (one 4-op DVE chain over 10 head-groups)
            # and ONE transpose kick per chunk, on the ScalarE (Act) HWDGE
            # ring so it never FIFOs behind the bulk input stream
            sb_qk = stage_q(pq)
            stage_kv(sb_qk, pkv, m)
            ms = slice(m * P, (m + 1) * P)
            qk_ro = rope_block(sb_qk, 0, HL + KVL, m)
            nc.scalar.dma_start_transpose(out=sb_qkT[:, :, ms], in_=qk_ro[:, :])

        # m=0, m=1 and m=2's q columns share each arriving k-chunk during
        # the DMA ramp (three chunks in flight across the split PSUM pools)
        pq0 = ps_qq.tile([P, QD], F32, tag="pq")
        pkv0 = ps_kv.tile([P, 2 * KD], F32, tag="pkv")
        pq1 = ps_qq.tile([P, QD], F32, tag="pq")
        pkv1 = ps_kv.tile([P, 2 * KD], F32, tag="pkv")
        pq2 = ps_qq.tile([P, QD], F32, tag="pq")
        for k in range(KC):
            proj_q(pq0, 0, k)
            proj_kv(pkv0, 0, k)
            proj_q(pq1, 1, k)
            proj_kv(pkv1, 1, k)
            mm = proj_q(pq2, 2, k)
        # release the wo load only once the input streaming has drained
        _add_dep_helper(wo_dma.ins, mm.ins,
                        reason="delay wo load past input ramp")
        finish_m(pq0, pkv0, 0)
        pkv2 = ps_kv.tile([P, 2 * KD], F32, tag="pkv")
        for k in range(KC):
            proj_kv(pkv2, 2, k)
        finish_m(pq1, pkv1, 1)
        prev = (pq2, pkv2, 2)
        for m in range(3, SC - 1):
            pq = ps_qq.tile([P, QD], F32, tag="pq")
            pkv = ps_kv.tile([P, 2 * KD], F32, tag="pkv")
            # q columns for all k first, kv columns after: the kv tile's
            # slot WAR (stage_kv two chunks back) has drained by then
            for k in range(KC):
                proj_q(pq, m, k)
            for k in range(KC):
                proj_kv(pkv, m, k)
            finish_m(*prev)
            prev = (pq, pkv, m)

        # Last chunk: q columns first so its qT (needed by phase B almost
        # immediately) lands while the PE still has kv-proj + early score
        # work; k/v columns follow and kT rides the sync ring in parallel.
        M7 = SC - 1
        pq7 = ps_qq.tile([P, QD], F32, tag="pq")
        for k in range(KC):
            proj_q(pq7, M7, k)
        finish_m(*prev)
        sb_qk7 = stage_q(pq7)
        ms7 = slice(M7 * P, (M7 + 1) * P)
        q7_ro = rope_block(sb_qk7, 0, HL, M7)
        # both last-chunk transposes ride the (by now idle) sync ring so
        # the Scalar FIFO can run stage_kv(7) + the early exps immediately
        nc.sync.dma_start_transpose(out=sb_qT[:, :, ms7], in_=q7_ro[:, :])
        pkv7 = ps_kv.tile([P, 2 * KD], F32, tag="pkv")
        for k in range(KC):
            proj_kv(pkv7, M7, k)
        stage_kv(sb_qk7, pkv7, M7)
        ps_kv.release()
        ps_qq.release()
        # Shared 4-slot [P,512] fp32 PSUM pool for score tiles, ones-matmul
        # sum tiles AND phase C's out_proj tiles; 2-slot [P,S] pool for the
        # paired heads' AV accumulators.
        ps_small = tc.alloc_tile_pool(name="ps_small", bufs=4, space="PSUM")
        ps_av = tc.alloc_tile_pool(name="ps_av", bufs=2, space="PSUM")

        exp_scale = float(1 / math.sqrt(D))
        # ragged probsT: row ki only stores the causal columns [ki*P, S)
        OFF = [0]
        for ki in range(SC):
            OFF.append(OFF[-1] + S - ki * P)
        PTOT = OFF[-1]          # 4608

        head_tiles = {}

        def get_head_tiles(h):
            if h not in head_tiles:
                probsT = battn.tile([P, PTOT], BF, tag="probsT", bufs=3,
                                    name=f"probsT{h}")
                acc = battn.tile([P, S], BF, tag="acc", bufs=3,
                                 name=f"acc{h}")
                head_tiles[h] = (probsT, acc)
            return head_tiles[h]

        def pt(probsT, ki, a, b):
            q0 = ki * P
            return probsT[:, OFF[ki] + (a - q0):OFF[ki] + (b - q0)]

        # early score pieces: cover the PE while the last chunk's RoPE +
        # transposes drain (they only touch token chunks 0-3)
        early_done = set()
        for h in (0, 1, 2):
            probsT, _ = get_head_tiles(h)
            for ki in range(4):
                a = ki * P
                psc = ps_small.tile([P, 512], F32, tag="ps", name="psce")
                nc.tensor.matmul(psc[:, 0:512 - a], sb_kT[:, 0, a:a + P],
                                 sb_qT[:, h, a:512], start=True, stop=True)
                nc.scalar.activation(pt(probsT, ki, a, 512),
                                     psc[:, 0:512 - a],
                                     Exp, scale=exp_scale)
                early_done.add((h, ki, a))

        k7_ro = rope_block(sb_qk7, QD, KVL, M7)
        nc.sync.dma_start_transpose(out=sb_kT[:, :, ms7], in_=k7_ro[:, :])
        ropepool.release()
        projpool.release()
        # out_proj partials for heads 0-3 (computed during B pairs 2-3)
        # live in the space the projection pools just freed
        partpool = tc.alloc_tile_pool(name="part", bufs=1)
        sb_part = partpool.tile([P, SC, HID], BF)

        # ---------------- Phase B: causal attention -------------------
        # Heads processed in pairs with interleaved k-chunks: one head's
        # exp/mask/acc chain hides under the other's score/AV matmuls.

        def head_ctx(h):
            g = h // (HL // KVL)
            probsT, acc = get_head_tiles(h)
            pav = ps_av.tile([P, S], F32, tag="pav")
            return (h, g, probsT, acc, pav)

        def pieces_ki(ctx, ki):
            h, g, probsT, acc, pav = ctx
            q0 = ki * P
            kslice = slice(q0, q0 + P)
            for (a, b) in _pieces(q0, S):
                if (h, ki, a) in early_done:
                    continue
                psc = ps_small.tile([P, 512], F32, tag="ps")
                nc.tensor.matmul(psc[:, 0:b - a],
                                 sb_kT[:, g, kslice],
                                 sb_qT[:, h, a:b],
                                 start=True, stop=True)
                nc.scalar.activation(pt(probsT, ki, a, b),
                                     psc[:, 0:b - a], Exp,
                                     scale=exp_scale)
            # mask strictly-below-diagonal of the diag block on GpSimd
            nc.gpsimd.tensor_mul(pt(probsT, ki, q0, q0 + P),
                                 pt(probsT, ki, q0, q0 + P),
                                 utmask[:, :])
            # accumulate the column sums on DVE (2x bf16)
            if ki == 0:
                nc.vector.tensor_copy(acc[:, :], pt(probsT, 0, 0, S))
            else:
                nc.vector.tensor_add(acc[:, q0:], acc[:, q0:],
                                     pt(probsT, ki, q0, S))

        def av_ki(ctx, ki):
            h, g, probsT, acc, pav = ctx
            st, sp = (ki == 0), (ki == SC - 1)
            for (a, b) in _pieces(ki * P, S):
                nc.tensor.matmul(pav[:, a:b],
                                 sb_v[:, ki, g * D:(g + 1) * D],
                                 pt(probsT, ki, a, b),
                                 start=st, stop=sp)

        def finalize(ctx):
            h, g, probsT, acc, pav = ctx
            av_ki(ctx, SC - 1)
            # ones-matrix matmul = column sums broadcast across partitions
            rbc = battn.tile([P, S], F32, tag="rbc", bufs=1)
            for (a, b) in _pieces(0, S):
                psbc = ps_small.tile([P, 512], F32, tag="ps")
                nc.tensor.matmul(psbc[:, 0:b - a], ones_mat[:, :],
                                 acc[:, a:b], start=True, stop=True)
                nc.vector.reciprocal_approx_fast(rbc[:, a:b],
                                                 psbc[:, 0:b - a])
            nc.vector.tensor_mul(sb_attnT[:, h, :], pav[:, :], rbc[:, :])

        def partial_block(m, nb):
            # out_proj contribution of heads 0-3 for block (m, nb), staged
            # to SBUF bf16: fills the PE while ScalarE paces the exp chain
            ms = slice(m * P, (m + 1) * P)
            nsl = slice(nb * 512, (nb + 1) * 512)
            py = ps_small.tile([P, 512], F32, tag="ps")
            for k in range(4):
                nc.tensor.matmul(py[:, :], sb_attnT[:, k, ms],
                                 sb_wo[:, k, nsl],
                                 start=(k == 0), stop=(k == 3))
            if (m + nb) % 2:
                nc.vector.tensor_copy(sb_part[:, m, nsl], py[:, :])
            else:
                nc.scalar.copy(sb_part[:, m, nsl], py[:, :])

        part_iter = iter([(m, nb) for m in range(SC)
                          for nb in range(HID // 512)])
        pending = [None]
        for hp in range(HL // 2):
            if pending[0] is not None:
                # finalize the previous pair before its pav slots rotate
                pending[0]()
                pending[0] = None
            ctxA = head_ctx(2 * hp)
            ctxB = head_ctx(2 * hp + 1)
            for ki in range(SC):
                pieces_ki(ctxA, ki)
                pieces_ki(ctxB, ki)
                if ki >= 1:
                    av_ki(ctxA, ki - 1)
                    av_ki(ctxB, ki - 1)
                if hp >= 2:
                    partial_block(*next(part_iter))
                    partial_block(*next(part_iter))

            def make_pending(cA, cB):
                def run():
                    finalize(cA)
                    finalize(cB)
                return run
            pending[0] = make_pending(ctxA, ctxB)

        # ---------------- Phase C: out projection ---------------------
        # Fine-grained: one 512-col PSUM slot per n-block with k-inner
        # accumulation.  m=0's first blocks run k<6 while the last pair's
        # finalize chains (attnT[6], attnT[7]) drain.
        ypool = tc.alloc_tile_pool(name="ysb", bufs=2)
        # the last pair finalizes first (before any phase-C ps_small allocs
        # so its ones-tiles don't rotate onto a held out_proj slot); m=0's
        # first blocks then run k=4,5 while attnT[6]/attnT[7] drain
        pending[0]()
        for m in range(SC):
            ms = slice(m * P, (m + 1) * P)
            last_m = (m == SC - 1)
            ysb = ypool.tile([P, HID], BF, tag="ysb")
            pys = {}
            for nb in range(HID // 512):
                nsl = slice(nb * 512, (nb + 1) * 512)
                py = ps_small.tile([P, 512], F32, tag="ps")
                if m == 0 and nb < 2:
                    pys[nb] = py
                    for k in (4, 5):
                        nc.tensor.matmul(py[:, :], sb_attnT[:, k, ms],
                                         sb_wo[:, k, nsl],
                                         start=(k == 4), stop=False)
                    if nb == 0:
                        continue
                    for pnb in (0, 1):
                        pnsl = slice(pnb * 512, (pnb + 1) * 512)
                        for k in (HL - 2, HL - 1):
                            nc.tensor.matmul(pys[pnb][:, :],
                                             sb_attnT[:, k, ms],
                                             sb_wo[:, k, pnsl],
                                             start=False, stop=(k == HL - 1))
                    nc.vector.tensor_add(ysb[:, 0:512], pys[0][:, :],
                                         sb_part[:, 0, 0:512])
                else:
                    for k in range(4, HL):
                        nc.tensor.matmul(py[:, :],
                                         sb_attnT[:, k, ms],
                                         sb_wo[:, k, nsl],
                                         start=(k == 4), stop=(k == HL - 1))
                # combine the heads 4-7 PSUM block with the heads 0-3
                # bf16 partial on the DVE
                nc.vector.tensor_add(ysb[:, nsl], py[:, :],
                                     sb_part[:, m, nsl])
                if nb % 2 == 1:
                    if not last_m:
                        # store per 1024-col pair
                        nc.sync.dma_start(
                            out=out[ms, nb * 512 - 512:nb * 512 + 512],
                            in_=ysb[:, nb * 512 - 512:nb * 512 + 512])
                    elif nb == 1:
                        nc.sync.dma_start(out=out[ms, 0:1024],
                                          in_=ysb[:, 0:1024])
                    else:
                        nc.sync.dma_start(out=out[ms, 1536:2048],
                                          in_=ysb[:, 1536:2048])
                elif last_m and nb == 2:
                    # tail: store this block early so the final kick
                    # only moves the last 128KB
                    nc.sync.dma_start(out=out[ms, 1024:1536],
                                      in_=ysb[:, 1024:1536])

        ypool.release()
        partpool.release()
        ps_av.release()
        ps_small.release()
        battn.release()
        qkvpool.release()
        wpool.release()
        cpool.release()

    nc.compile()
    return nc


def _get_nc():
    if "nc" not in _NC_CACHE:
        _NC_CACHE["nc"] = build_nc()
    return _NC_CACHE["nc"]


def _chunk_major(a, nchunks):
    """[nchunks*128, cols] -> [128, nchunks*cols] partition-major layout."""
    n = a.shape[1]
    return np.ascontiguousarray(
        a.reshape(nchunks, P, n).transpose(1, 0, 2).reshape(P, nchunks * n))


def _make_in_maps(x, cos, sin, wq, wk, wv, wo):
    bf = ml_dtypes.bfloat16
    HALF = D // 2
    sin_rot = np.concatenate([-sin[:, :HALF], sin[:, HALF:]], axis=1)
    cos_t = _chunk_major(cos, SC).astype(bf)
    sin_t = _chunk_major(sin_rot, SC).astype(bf)
    in_maps = []
    for core in range(NCORES):
        b, t = divmod(core, TP)
        wqkv = np.concatenate([
            wq[:, t * QD:(t + 1) * QD],
            wk[:, t * KD:(t + 1) * KD],
            wv[:, t * KD:(t + 1) * KD],
        ], axis=1)
        in_maps.append({
            "xT": _chunk_major(np.ascontiguousarray(x[b].T), KC).astype(bf),
            "wqkv": _chunk_major(wqkv, KC).astype(bf),
            "wo": _chunk_major(wo[t * QD:(t + 1) * QD, :], HL).astype(bf),
            "cos_t": cos_t, "sin_t": sin_t,
        })
    return in_maps


def run(inputs, trace=False):
    if trace:
        _ensure_ntff_hook()
    nc = _get_nc()
    in_maps = _make_in_maps(
        np.asarray(inputs["x"], np.float32),
        np.asarray(inputs["cos"], np.float32),
        np.asarray(inputs["sin"], np.float32),
        np.asarray(inputs["wq"], np.float32),
        np.asarray(inputs["wk"], np.float32),
        np.asarray(inputs["wv"], np.float32),
        np.asarray(inputs["wo"], np.float32),
    )
    try:
        res = run_bass_kernel_spmd(nc, in_maps, list(range(NCORES)),
                                   trace=trace)
    except Exception:
        # one retry: a previous process can leave a core wedged
        res = run_bass_kernel_spmd(nc, in_maps, list(range(NCORES)),
                                   trace=trace)
    outs = [np.asarray(r["out"]).astype(np.float32) for r in res.results]
    y = np.stack([outs[TP * b] + outs[TP * b + 1] for b in range(B)])
    return y, res


def kernel(**inputs):
    y, _ = run(inputs, trace=False)
    return y


# revision 21
# speedup vs baseline: 1.1127x; 1.0022x over previous
"""GQA causal attention with RoPE, distributed over 8 trn2 NeuronCores.

Sharding: 4-way data parallel over batch x 2-way tensor parallel over heads.
Core c = 2*b + t handles batch b with query heads [t*8, (t+1)*8) and KV heads
[t*2, (t+1)*2).  Each core computes a row-sharded out_proj partial; the pair
partials are summed on the host during unsharding.

On-chip algorithm (per core, bf16 matmuls / fp32 softmax):
  1. QKV projections from host-prearranged partition-major inputs
     (xT/wqkv/wo stored as [128, chunks*cols] so every DMA descriptor is a
     full 2-chunk 4-6KB partition line; the first chunk is split across
     4 partition-group DMAs so the first matmul starts ~1.5us after the
     queues open).  Token chunks m=0,1 and m=2's q-columns are projected
     k-outer while the input streams; the PSUM pools are split
     (q [P,1024] bufs=3 / kv [P,512] bufs=2) so three chunks are in
     flight during the DMA ramp.
  2. Each projection chunk is staged PSUM->SBUF (bf16) on ScalarE; RoPE
     runs all-bf16 on the DVE (2x perf mode).  The q transposes are issued
     on the ScalarE HWDGE ring and the k transposes on the sync ring so
     they never FIFO behind the bulk input stream, and the last chunk
     projects its q columns first so phase B's qT dependency lands before
     the early score pieces run out.
  3. Scores computed TRANSPOSED (scoresT[k_tok, q_tok]) so no probs
     transpose is needed: exp on ScalarE, column sums accumulated on DVE,
     summed across partitions by a ones-matmul, AV matmul consumes probsT
     directly, normalization happens once on the attention output.
  4. Causality: blocks with ki > qi are never computed; the diagonal block
     is masked with a precomputed upper-triangular 0/1 mask after exp.
     Heads are processed in PAIRS with interleaved k-chunks so the
     score->exp->AV chain of one head hides under the other's matmuls
     (phase B paces on ScalarE exp throughput).
  5. out_proj from feature-major attnT with wo as the moving operand,
     fine-grained (one 512-col PSUM bank per block, k-inner accumulation).
     The last chunk's output stores are split across partition groups /
     queues so the tail after the final matmul is ~1.5us.
"""

import math
import sys

sys.path.insert(0, "/opt/trn_rl_repo")

import ml_dtypes
import numpy as np

import concourse.bacc as bacc
import concourse.mybir as mybir
import concourse.tile as tile
from concourse.bass import _add_dep_helper
from concourse.bass_utils import run_bass_kernel_spmd
from concourse.masks import make_upper_triangular

B, S, HID = 4, 1024, 2048
H, KV, D = 16, 4, 128
P = 128
TP = 2                  # tensor-parallel ways (head split)
HL = H // TP            # 8 query heads per core
KVL = KV // TP          # 2 kv heads per core
QD = HL * D             # 1024
KD = KVL * D            # 256
SC = S // P             # 8 token chunks
KC = HID // P           # 16 hidden chunks
QKVD = QD + 2 * KD      # 1536 = q 1024 | k 256 | v 256
NCORES = 8
BF = mybir.dt.bfloat16
F32 = mybir.dt.float32
Exp = mybir.ActivationFunctionType.Exp

_NC_CACHE = {}


def _ensure_ntff_hook():
    """The agent image's antenv lacks axon_hooks, so bass_utils' trace=True
    path can't find the NTFF profile hook trn_boot would have registered.
    Recreate the module and register the ctypes-based hook ourselves."""
    try:
        from antenv.axon_hooks import get_axon_ntff_profile_hook  # noqa: F401
        return
    except ImportError:
        pass
    import types

    import antenv

    mod = types.ModuleType("antenv.axon_hooks")
    _state = {"hook": None}
    mod.set_axon_ntff_profile_hook = lambda h: _state.__setitem__("hook", h)
    mod.get_axon_ntff_profile_hook = lambda: _state["hook"]
    sys.modules["antenv.axon_hooks"] = mod
    antenv.axon_hooks = mod
    try:
        from trn_agent_boot.trn_boot import _ntff_profile_via_ctypes

        hook = _ntff_profile_via_ctypes("/opt/axon/libaxon_pjrt.so")
        if hook is not None:
            mod.set_axon_ntff_profile_hook(hook)
    except Exception as e:  # pragma: no cover
        print(f"NTFF hook registration failed: {e}", file=sys.stderr)


def _pieces(start, end, step=512):
    """Split [start, end) into spans of at most `step`, aligned so no span
    crosses a `step` boundary (PSUM: one bank per matmul)."""
    out = []
    a = start
    while a < end:
        b = min((a // step + 1) * step, end)
        out.append((a, b))
        a = b
    return out


def build_nc():
    nc = bacc.Bacc("TRN2", target_bir_lowering=False, debug=False,
                   num_devices=NCORES)

    # chunk-contiguous DRAM layouts: each per-chunk DMA reads a sequential
    # 256-384KB block (strided partition-major layouts measured ~25% slower
    # HBM throughput)
    xT = nc.declare_dram_parameter("xT", [HID, S], BF, isOutput=False)
    wqkv = nc.declare_dram_parameter("wqkv", [HID, QKVD], BF, isOutput=False)
    wo = nc.declare_dram_parameter("wo", [QD, HID], BF, isOutput=False)
    cos_t = nc.declare_dram_parameter("cos_t", [P, SC * D], BF, isOutput=False)
    sin_t = nc.declare_dram_parameter("sin_t", [P, SC * D], BF, isOutput=False)
    out = nc.declare_dram_parameter("out", [S, HID], BF, isOutput=True)

    with tile.TileContext(nc) as tc:
        # ---- persistent pools (allocated first: fixed addresses) ----
        cpool = tc.alloc_tile_pool(name="consts", bufs=1)
        wpool = tc.alloc_tile_pool(name="wpool", bufs=1)
        qkvpool = tc.alloc_tile_pool(name="qkvpool", bufs=1)
        # phase B's SBUF pool allocated BEFORE phase A's pools so their
        # address ranges are disjoint: no release barrier between A and B.
        battn = tc.alloc_tile_pool(name="battn", bufs=2)

        utmask = cpool.tile([P, P], BF)
        ones_mat = cpool.tile([P, P], BF)

        sb_wo = wpool.tile([P, HL, HID], BF)

        # combined feature-major q+k so each chunk's RoPE output transposes
        # with a single DMA kick: groups 0..7 = q heads, 8..9 = k heads
        sb_qkT = qkvpool.tile([P, HL + KVL, S], BF)
        sb_qT = sb_qkT[:, 0:HL, :]
        sb_kT = sb_qkT[:, HL:HL + KVL, :]
        sb_v = qkvpool.tile([P, SC, KD], BF)      # token-major v
        sb_attnT = qkvpool.tile([P, HL, S], BF)   # feature-major attn out

        # ---------------- Phase A: projections + RoPE -----------------
        projpool = tc.alloc_tile_pool(name="proj", bufs=1)
        ropepool = tc.alloc_tile_pool(name="rope", bufs=2)
        # split projection PSUM: q-part [P,1024] (2 banks) x3, kv-part
        # [P,512] (1 bank) x2 -> 8 banks, three chunks in flight at ramp
        ps_qq = tc.alloc_tile_pool(name="ps_qq", bufs=3, space="PSUM")
        ps_kv = tc.alloc_tile_pool(name="ps_kv", bufs=2, space="PSUM")

        sb_xT = projpool.tile([P, KC, S], BF)
        sb_wqkv = projpool.tile([P, KC, QKVD], BF)

        xT_r = xT.rearrange("(c p) s -> p c s", p=P)
        wqkv_r = wqkv.rearrange("(c p) n -> p c n", p=P)
        # chunk 0 split fine so the very first matmul only waits on ~230 KB
        nc.sync.dma_start(out=sb_wqkv[:, 0, 0:512], in_=wqkv_r[:, 0, 0:512])
        nc.sync.dma_start(out=sb_xT[:, 0, 0:384], in_=xT_r[:, 0, 0:384])
        nc.sync.dma_start(out=sb_wqkv[:, 0, 512:QKVD],
                          in_=wqkv_r[:, 0, 512:QKVD])
        nc.sync.dma_start(out=sb_xT[:, 0, 384:S], in_=xT_r[:, 0, 384:S])
        nc.sync.dma_start(out=sb_wqkv[:, 1, :], in_=wqkv_r[:, 1, :])
        nc.sync.dma_start(out=sb_xT[:, 1, :], in_=xT_r[:, 1, :])
        sb_ck = projpool.tile([P, SC, D], BF)
        nc.sync.dma_start(out=sb_ck[:, :, :], in_=cos_t[:, :])
        sb_sk = projpool.tile([P, SC, D], BF)
        nc.sync.dma_start(out=sb_sk[:, :, :], in_=sin_t[:, :])
        for c in range(2, KC):
            nc.sync.dma_start(out=sb_wqkv[:, c, :], in_=wqkv_r[:, c, :])
            nc.sync.dma_start(out=sb_xT[:, c, :], in_=xT_r[:, c, :])
        # wo is only needed in phase C: delay its (4 MB) load until the
        # input streaming has drained (dep added below)
        wo_dma = nc.sync.dma_start(
            out=sb_wo[:, :, :],
            in_=wo.rearrange("(c p) n -> p c n", p=P))

        # mask/ones builders issued after the DMA starts so the sync engine
        # kicks off the input stream first (they are not needed until B)
        make_upper_triangular(nc, utmask[:, :], val=1.0, diag=True)
        nc.vector.memset(ones_mat[:, :], 1.0)

        HALF = D // 2

        def rope_block(sb_src, lo, nh, m):
            """RoPE `nh` consecutive heads of the staged bf16 chunk (cols
            [lo, lo+nh*D)) in one batched op per step, via free-dim-broadcast
            cos/sin APs.  All-bf16 so the DVE runs in 2x mode.  Returns a
            bf16 SBUF tile [P, nh*D]."""
            # t1 is produced+consumed inside one in-order DVE chain: bufs=1
            t1 = ropepool.tile([P, nh, D], BF, tag="t1", bufs=1)
            ro = ropepool.tile([P, nh * D], BF, tag="ro", bufs=2)
            src = sb_src[:, lo:lo + nh * D].rearrange("p (h d) -> p h d", h=nh)
            sin_lo = sb_sk[:, m:m + 1, 0:HALF].broadcast_to([P, nh, HALF])
            sin_hi = sb_sk[:, m:m + 1, HALF:D].broadcast_to([P, nh, HALF])
            cos_b = sb_ck[:, m:m + 1, :].broadcast_to([P, nh, D])
            # rot_half * sin (sin table pre-negated on first half)
            nc.vector.tensor_mul(t1[:, :, 0:HALF], src[:, :, HALF:D], sin_lo)
            nc.vector.tensor_mul(t1[:, :, HALF:D], src[:, :, 0:HALF], sin_hi)
            ror = ro[:, :].rearrange("p (h d) -> p h d", h=nh)
            # ro = src*cos + t1
            nc.vector.tensor_mul(ror, src, cos_b)
            nc.vector.tensor_add(ror, ror, t1[:, :, :])
            return ro

        def proj_q(pq, m, k):
            st, sp = (k == 0), (k == KC - 1)
            lhsT = sb_xT[:, k, m * P:(m + 1) * P]
            for n in (0, 1):
                mm = nc.tensor.matmul(
                    pq[:, n * 512:(n + 1) * 512], lhsT,
                    sb_wqkv[:, k, n * 512:(n + 1) * 512],
                    start=st, stop=sp)
            return mm

        def proj_kv(pkv, m, k):
            st, sp = (k == 0), (k == KC - 1)
            lhsT = sb_xT[:, k, m * P:(m + 1) * P]
            return nc.tensor.matmul(
                pkv[:, :], lhsT, sb_wqkv[:, k, 1024:QKVD],
                start=st, stop=sp)

        def stage_q(pq):
            sb_qk = ropepool.tile([P, QD + KD], BF, tag="qk")
            nc.scalar.copy(sb_qk[:, 0:512], pq[:, 0:512])
            nc.scalar.copy(sb_qk[:, 512:QD], pq[:, 512:QD])
            return sb_qk

        def stage_kv(sb_qk, pkv, m):
            nc.scalar.copy(sb_qk[:, QD:QD + KD], pkv[:, 0:KD])
            nc.scalar.copy(sb_v[:, m, :], pkv[:, KD:2 * KD])

        def finish_m(pq, pkv, m):
            # combined q+k RoPE (one 4-op DVE chain over 10 head-groups)
            # and ONE transpose kick per chunk, on the ScalarE (Act) HWDGE
            # ring so it never FIFOs behind the bulk input stream
            sb_qk = stage_q(pq)
            stage_kv(sb_qk, pkv, m)
            ms = slice(m * P, (m + 1) * P)
            qk_ro = rope_block(sb_qk, 0, HL + KVL, m)
            nc.scalar.dma_start_transpose(out=sb_qkT[:, :, ms], in_=qk_ro[:, :])

        # m=0, m=1 and m=2's q columns share each arriving k-chunk during
        # the DMA ramp (three chunks in flight across the split PSUM pools)
        pq0 = ps_qq.tile([P, QD], F32, tag="pq")
        pkv0 = ps_kv.tile([P, 2 * KD], F32, tag="pkv")
        pq1 = ps_qq.tile([P, QD], F32, tag="pq")
        pkv1 = ps_kv.tile([P, 2 * KD], F32, tag="pkv")
        pq2 = ps_qq.tile([P, QD], F32, tag="pq")
        for k in range(KC):
            proj_q(pq0, 0, k)
            proj_kv(pkv0, 0, k)
            proj_q(pq1, 1, k)
            proj_kv(pkv1, 1, k)
            mm = proj_q(pq2, 2, k)
        # release the wo load only once the input streaming has drained
        _add_dep_helper(wo_dma.ins, mm.ins,
                        reason="delay wo load past input ramp")
        finish_m(pq0, pkv0, 0)
        pkv2 = ps_kv.tile([P, 2 * KD], F32, tag="pkv")
        for k in range(KC):
            proj_kv(pkv2, 2, k)
        finish_m(pq1, pkv1, 1)
        prev = (pq2, pkv2, 2)
        for m in range(3, SC - 1):
            pq = ps_qq.tile([P, QD], F32, tag="pq")
            pkv = ps_kv.tile([P, 2 * KD], F32, tag="pkv")
            # q columns for all k first, kv columns after: the kv tile's
            # slot WAR (stage_kv two chunks back) has drained by then
            for k in range(KC):
                proj_q(pq, m, k)
            for k in range(KC):
                proj_kv(pkv, m, k)
            finish_m(*prev)
            prev = (pq, pkv, m)

        # Last chunk: q columns first so its qT (needed by phase B almost
        # immediately) lands while the PE still has kv-proj + early score
        # work; k/v columns follow and kT rides the sync ring in parallel.
        M7 = SC - 1
        pq7 = ps_qq.tile([P, QD], F32, tag="pq")
        for k in range(KC):
            proj_q(pq7, M7, k)
        finish_m(*prev)
        sb_qk7 = stage_q(pq7)
        ms7 = slice(M7 * P, (M7 + 1) * P)
        q7_ro = rope_block(sb_qk7, 0, HL, M7)
        # both last-chunk transposes ride the (by now idle) sync ring so
        # the Scalar FIFO can run stage_kv(7) + the early exps immediately
        nc.sync.dma_start_transpose(out=sb_qT[:, :, ms7], in_=q7_ro[:, :])
        pkv7 = ps_kv.tile([P, 2 * KD], F32, tag="pkv")
        for k in range(KC):
            proj_kv(pkv7, M7, k)
        stage_kv(sb_qk7, pkv7, M7)
        ps_kv.release()
        ps_qq.release()
        # Shared 4-slot [P,512] fp32 PSUM pool for score tiles, ones-matmul
        # sum tiles AND phase C's out_proj tiles; 2-slot [P,S] pool for the
        # paired heads' AV accumulators.
        ps_small = tc.alloc_tile_pool(name="ps_small", bufs=4, space="PSUM")
        ps_av = tc.alloc_tile_pool(name="ps_av", bufs=2, space="PSUM")

        exp_scale = float(1 / math.sqrt(D))
        # ragged probsT: row ki only stores the causal columns [ki*P, S)
        OFF = [0]
        for ki in range(SC):
            OFF.append(OFF[-1] + S - ki * P)
        PTOT = OFF[-1]          # 4608

        head_tiles = {}

        def get_head_tiles(h):
            if h not in head_tiles:
                probsT = battn.tile([P, PTOT], BF, tag="probsT", bufs=3,
                                    name=f"probsT{h}")
                acc = battn.tile([P, S], BF, tag="acc", bufs=3,
                                 name=f"acc{h}")
                head_tiles[h] = (probsT, acc)
            return head_tiles[h]

        def pt(probsT, ki, a, b):
            q0 = ki * P
            return probsT[:, OFF[ki] + (a - q0):OFF[ki] + (b - q0)]

        # early score pieces: cover the PE while the last chunk's RoPE +
        # transposes drain (they only touch token chunks 0-3)
        early_done = set()
        for h in (0, 1, 2):
            probsT, _ = get_head_tiles(h)
            for ki in range(4):
                a = ki * P
                psc = ps_small.tile([P, 512], F32, tag="ps", name="psce")
                nc.tensor.matmul(psc[:, 0:512 - a], sb_kT[:, 0, a:a + P],
                                 sb_qT[:, h, a:512], start=True, stop=True)
                nc.scalar.activation(pt(probsT, ki, a, 512),
                                     psc[:, 0:512 - a],
                                     Exp, scale=exp_scale)
                early_done.add((h, ki, a))

        k7_ro = rope_block(sb_qk7, QD, KVL, M7)
        nc.sync.dma_start_transpose(out=sb_kT[:, :, ms7], in_=k7_ro[:, :])
        ropepool.release()
        projpool.release()
        # out_proj partials for heads 0-3 (computed during B pairs 2-3)
        # live in the space the projection pools just freed
        partpool = tc.alloc_tile_pool(name="part", bufs=1)
        sb_part = partpool.tile([P, SC, HID], BF)

        # ---------------- Phase B: causal attention -------------------
        # Heads processed in pairs with interleaved k-chunks: one head's
        # exp/mask/acc chain hides under the other's score/AV matmuls.

        def head_ctx(h):
            g = h // (HL // KVL)
            probsT, acc = get_head_tiles(h)
            pav = ps_av.tile([P, S], F32, tag="pav")
            return (h, g, probsT, acc, pav)

        def pieces_ki(ctx, ki):
            h, g, probsT, acc, pav = ctx
            q0 = ki * P
            kslice = slice(q0, q0 + P)
            for (a, b) in _pieces(q0, S):
                if (h, ki, a) in early_done:
                    continue
                psc = ps_small.tile([P, 512], F32, tag="ps")
                nc.tensor.matmul(psc[:, 0:b - a],
                                 sb_kT[:, g, kslice],
                                 sb_qT[:, h, a:b],
                                 start=True, stop=True)
                nc.scalar.activation(pt(probsT, ki, a, b),
                                     psc[:, 0:b - a], Exp,
                                     scale=exp_scale)
            # mask strictly-below-diagonal of the diag block on GpSimd
            nc.gpsimd.tensor_mul(pt(probsT, ki, q0, q0 + P),
                                 pt(probsT, ki, q0, q0 + P),
                                 utmask[:, :])
            # accumulate the column sums on DVE (2x bf16)
            if ki == 0:
                nc.vector.tensor_copy(acc[:, :], pt(probsT, 0, 0, S))
            else:
                nc.vector.tensor_add(acc[:, q0:], acc[:, q0:],
                                     pt(probsT, ki, q0, S))

        def av_ki(ctx, ki):
            h, g, probsT, acc, pav = ctx
            st, sp = (ki == 0), (ki == SC - 1)
            for (a, b) in _pieces(ki * P, S):
                nc.tensor.matmul(pav[:, a:b],
                                 sb_v[:, ki, g * D:(g + 1) * D],
                                 pt(probsT, ki, a, b),
                                 start=st, stop=sp)

        def finalize(ctx):
            h, g, probsT, acc, pav = ctx
            av_ki(ctx, SC - 1)
            # ones-matrix matmul = column sums broadcast across partitions
            rbc = battn.tile([P, S], F32, tag="rbc", bufs=1)
            for (a, b) in _pieces(0, S):
                psbc = ps_small.tile([P, 512], F32, tag="ps")
                nc.tensor.matmul(psbc[:, 0:b - a], ones_mat[:, :],
                                 acc[:, a:b], start=True, stop=True)
                nc.vector.reciprocal_approx_fast(rbc[:, a:b],
                                                 psbc[:, 0:b - a])
            nc.vector.tensor_mul(sb_attnT[:, h, :], pav[:, :], rbc[:, :])

        def partial_block(m, nb):
            # out_proj contribution of heads 0-3 for block (m, nb), staged
            # to SBUF bf16: fills the PE while ScalarE paces the exp chain
            ms = slice(m * P, (m + 1) * P)
            nsl = slice(nb * 512, (nb + 1) * 512)
            py = ps_small.tile([P, 512], F32, tag="ps")
            for k in range(4):
                nc.tensor.matmul(py[:, :], sb_attnT[:, k, ms],
                                 sb_wo[:, k, nsl],
                                 start=(k == 0), stop=(k == 3))
            if (m + nb) % 2:
                nc.vector.tensor_copy(sb_part[:, m, nsl], py[:, :])
            else:
                nc.scalar.copy(sb_part[:, m, nsl], py[:, :])

        part_iter = iter([(m, nb) for m in range(SC)
                          for nb in range(HID // 512)])
        pending = [None]
        for hp in range(HL // 2):
            if pending[0] is not None:
                # finalize the previous pair before its pav slots rotate
                pending[0]()
                pending[0] = None
            ctxA = head_ctx(2 * hp)
            ctxB = head_ctx(2 * hp + 1)
            for ki in range(SC):
                pieces_ki(ctxA, ki)
                pieces_ki(ctxB, ki)
                if ki >= 1:
                    av_ki(ctxA, ki - 1)
                    av_ki(ctxB, ki - 1)
                if hp >= 2:
                    partial_block(*next(part_iter))
                    partial_block(*next(part_iter))

            def make_pending(cA, cB):
                def run():
                    finalize(cA)
                    finalize(cB)
                return run
            pending[0] = make_pending(ctxA, ctxB)

        # ---------------- Phase C: out projection ---------------------
        # Fine-grained: one 512-col PSUM slot per n-block with k-inner
        # accumulation.  m=0's first blocks run k<6 while the last pair's
        # finalize chains (attnT[6], attnT[7]) drain.
        ypool = tc.alloc_tile_pool(name="ysb", bufs=2)
        # the last pair finalizes first (before any phase-C ps_small allocs
        # so its ones-tiles don't rotate onto a held out_proj slot); m=0's
        # first blocks then run k=4,5 while attnT[6]/attnT[7] drain
        pending[0]()
        for m in range(SC):
            ms = slice(m * P, (m + 1) * P)
            last_m = (m == SC - 1)
            ysb = ypool.tile([P, HID], BF, tag="ysb")
            pys = {}
            for nb in range(HID // 512):
                nsl = slice(nb * 512, (nb + 1) * 512)
                py = ps_small.tile([P, 512], F32, tag="ps")
                if m == 0 and nb < 2:
                    pys[nb] = py
                    for k in (4, 5):
                        nc.tensor.matmul(py[:, :], sb_attnT[:, k, ms],
                                         sb_wo[:, k, nsl],
                                         start=(k == 4), stop=False)
                    if nb == 0:
                        continue
                    for pnb in (0, 1):
                        pnsl = slice(pnb * 512, (pnb + 1) * 512)
                        for k in (HL - 2, HL - 1):
                            nc.tensor.matmul(pys[pnb][:, :],
                                             sb_attnT[:, k, ms],
                                             sb_wo[:, k, pnsl],
                                             start=False, stop=(k == HL - 1))
                    nc.vector.tensor_add(ysb[:, 0:512], pys[0][:, :],
                                         sb_part[:, 0, 0:512])
                else:
                    for k in range(4, HL):
                        nc.tensor.matmul(py[:, :],
                                         sb_attnT[:, k, ms],
                                         sb_wo[:, k, nsl],
                                         start=(k == 4), stop=(k == HL - 1))
                # combine the heads 4-7 PSUM block with the heads 0-3
                # bf16 partial on the DVE
                nc.vector.tensor_add(ysb[:, nsl], py[:, :],
                                     sb_part[:, m, nsl])
                if nb % 2 == 1:
                    if not last_m:
                        # store per 1024-col pair
                        nc.sync.dma_start(
                            out=out[ms, nb * 512 - 512:nb * 512 + 512],
                            in_=ysb[:, nb * 512 - 512:nb * 512 + 512])
                    elif nb == 1:
                        nc.sync.dma_start(out=out[ms, 0:1024],
                                          in_=ysb[:, 0:1024])
                    else:
                        nc.sync.dma_start(out=out[ms, 1536:2048],
                                          in_=ysb[:, 1536:2048])
                elif last_m and nb == 2:
                    # tail: store this block early so the final kick
                    # only moves the last 128KB
                    nc.sync.dma_start(out=out[ms, 1024:1536],
                                      in_=ysb[:, 1024:1536])

        ypool.release()
        partpool.release()
        ps_av.release()
        ps_small.release()
        battn.release()
        qkvpool.release()
        wpool.release()
        cpool.release()

    nc.compile()
    return nc


def _get_nc():
    if "nc" not in _NC_CACHE:
        _NC_CACHE["nc"] = build_nc()
    return _NC_CACHE["nc"]


def _chunk_major(a, nchunks):
    """[nchunks*128, cols] -> [128, nchunks*cols] partition-major layout."""
    n = a.shape[1]
    return np.ascontiguousarray(
        a.reshape(nchunks, P, n).transpose(1, 0, 2).reshape(P, nchunks * n))


def _make_in_maps(x, cos, sin, wq, wk, wv, wo):
    bf = ml_dtypes.bfloat16
    HALF = D // 2
    sin_rot = np.concatenate([-sin[:, :HALF], sin[:, HALF:]], axis=1)
    cos_t = _chunk_major(cos, SC).astype(bf)
    sin_t = _chunk_major(sin_rot, SC).astype(bf)
    in_maps = []
    for core in range(NCORES):
        b, t = divmod(core, TP)
        wqkv = np.concatenate([
            wq[:, t * QD:(t + 1) * QD],
            wk[:, t * KD:(t + 1) * KD],
            wv[:, t * KD:(t + 1) * KD],
        ], axis=1)
        in_maps.append({
            "xT": np.ascontiguousarray(x[b].T).astype(bf),
            "wqkv": np.ascontiguousarray(wqkv).astype(bf),
            "wo": np.ascontiguousarray(wo[t * QD:(t + 1) * QD, :]).astype(bf),
            "cos_t": cos_t, "sin_t": sin_t,
        })
    return in_maps


def run(inputs, trace=False):
    if trace:
        _ensure_ntff_hook()
    nc = _get_nc()
    in_maps = _make_in_maps(
        np.asarray(inputs["x"], np.float32),
        np.asarray(inputs["cos"], np.float32),
        np.asarray(inputs["sin"], np.float32),
        np.asarray(inputs["wq"], np.float32),
        np.asarray(inputs["wk"], np.float32),
        np.asarray(inputs["wv"], np.float32),
        np.asarray(inputs["wo"], np.float32),
    )
    try:
        res = run_bass_kernel_spmd(nc, in_maps, list(range(NCORES)),
                                   trace=trace)
    except Exception:
        # one retry: a previous process can leave a core wedged
        res = run_bass_kernel_spmd(nc, in_maps, list(range(NCORES)),
                                   trace=trace)
    outs = [np.asarray(r["out"]).astype(np.float32) for r in res.results]
    y = np.stack([outs[TP * b] + outs[TP * b + 1] for b in range(B)])
    return y, res


def kernel(**inputs):
    y, _ = run(inputs, trace=False)
    return y


# revision 28
# speedup vs baseline: 1.1469x; 1.0308x over previous
"""GQA causal attention with RoPE, distributed over 8 trn2 NeuronCores.

Sharding: 4-way data parallel over batch x 2-way tensor parallel over heads.
Core c = 2*b + t handles batch b with query heads [t*8, (t+1)*8) and KV heads
[t*2, (t+1)*2).  Each core computes a row-sharded out_proj partial; the pair
partials are summed on the host during unsharding.

On-chip algorithm (per core, bf16 matmuls / fp32 softmax):
  1. QKV projections from host-prearranged partition-major inputs
     (xT/wqkv/wo stored as [128, chunks*cols] so every DMA descriptor is a
     full 2-chunk 4-6KB partition line; the first chunk is split across
     4 partition-group DMAs so the first matmul starts ~1.5us after the
     queues open).  Token chunks m=0,1 and m=2's q-columns are projected
     k-outer while the input streams; the PSUM pools are split
     (q [P,1024] bufs=3 / kv [P,512] bufs=2) so three chunks are in
     flight during the DMA ramp.
  2. Each projection chunk is staged PSUM->SBUF (bf16) on ScalarE; RoPE
     runs all-bf16 on the DVE (2x perf mode).  The q transposes are issued
     on the ScalarE HWDGE ring and the k transposes on the sync ring so
     they never FIFO behind the bulk input stream, and the last chunk
     projects its q columns first so phase B's qT dependency lands before
     the early score pieces run out.
  3. Scores computed TRANSPOSED (scoresT[k_tok, q_tok]) so no probs
     transpose is needed: exp on ScalarE, column sums accumulated on DVE,
     summed across partitions by a ones-matmul, AV matmul consumes probsT
     directly, normalization happens once on the attention output.
  4. Causality: blocks with ki > qi are never computed; the diagonal block
     is masked with a precomputed upper-triangular 0/1 mask after exp.
     Heads are processed in PAIRS with interleaved k-chunks so the
     score->exp->AV chain of one head hides under the other's matmuls
     (phase B paces on ScalarE exp throughput).
  5. out_proj from feature-major attnT with wo as the moving operand,
     fine-grained (one 512-col PSUM bank per block, k-inner accumulation).
     The last chunk's output stores are split across partition groups /
     queues so the tail after the final matmul is ~1.5us.
"""

import math
import sys

sys.path.insert(0, "/opt/trn_rl_repo")

import ml_dtypes
import numpy as np

import concourse.bacc as bacc
import concourse.mybir as mybir
import concourse.tile as tile
from concourse.bass import _add_dep_helper
from concourse.bass_utils import run_bass_kernel_spmd
from concourse.masks import make_upper_triangular

B, S, HID = 4, 1024, 2048
H, KV, D = 16, 4, 128
P = 128
TP = 2                  # tensor-parallel ways (head split)
HL = H // TP            # 8 query heads per core
KVL = KV // TP          # 2 kv heads per core
QD = HL * D             # 1024
KD = KVL * D            # 256
SC = S // P             # 8 token chunks
KC = HID // P           # 16 hidden chunks
QKVD = QD + 2 * KD      # 1536 = q 1024 | k 256 | v 256
NCORES = 8
BF = mybir.dt.bfloat16
F32 = mybir.dt.float32
Exp = mybir.ActivationFunctionType.Exp

_NC_CACHE = {}


def _ensure_ntff_hook():
    """The agent image's antenv lacks axon_hooks, so bass_utils' trace=True
    path can't find the NTFF profile hook trn_boot would have registered.
    Recreate the module and register the ctypes-based hook ourselves."""
    try:
        from antenv.axon_hooks import get_axon_ntff_profile_hook  # noqa: F401
        return
    except ImportError:
        pass
    import types

    import antenv

    mod = types.ModuleType("antenv.axon_hooks")
    _state = {"hook": None}
    mod.set_axon_ntff_profile_hook = lambda h: _state.__setitem__("hook", h)
    mod.get_axon_ntff_profile_hook = lambda: _state["hook"]
    sys.modules["antenv.axon_hooks"] = mod
    antenv.axon_hooks = mod
    try:
        from trn_agent_boot.trn_boot import _ntff_profile_via_ctypes

        hook = _ntff_profile_via_ctypes("/opt/axon/libaxon_pjrt.so")
        if hook is not None:
            mod.set_axon_ntff_profile_hook(hook)
    except Exception as e:  # pragma: no cover
        print(f"NTFF hook registration failed: {e}", file=sys.stderr)


def _pieces(start, end, step=512):
    """Split [start, end) into spans of at most `step`, aligned so no span
    crosses a `step` boundary (PSUM: one bank per matmul)."""
    out = []
    a = start
    while a < end:
        b = min((a // step + 1) * step, end)
        out.append((a, b))
        a = b
    return out


def build_nc():
    nc = bacc.Bacc("TRN2", target_bir_lowering=False, debug=False,
                   num_devices=NCORES)

    # chunk-contiguous DRAM layouts: each per-chunk DMA reads a sequential
    # 256-384KB block (strided partition-major layouts measured ~25% slower
    # HBM throughput)
    xT = nc.declare_dram_parameter("xT", [HID, S], BF, isOutput=False)
    wqkv = nc.declare_dram_parameter("wqkv", [HID, QKVD], BF, isOutput=False)
    wo = nc.declare_dram_parameter("wo", [QD, HID], BF, isOutput=False)
    cos_t = nc.declare_dram_parameter("cos_t", [P, SC * D], BF, isOutput=False)
    sin_t = nc.declare_dram_parameter("sin_t", [P, SC * D], BF, isOutput=False)
    out = nc.declare_dram_parameter("out", [S, HID], BF, isOutput=True)

    with tile.TileContext(nc) as tc:
        # ---- persistent pools (allocated first: fixed addresses) ----
        cpool = tc.alloc_tile_pool(name="consts", bufs=1)
        wpool = tc.alloc_tile_pool(name="wpool", bufs=1)
        qkvpool = tc.alloc_tile_pool(name="qkvpool", bufs=1)
        # phase B's SBUF pool allocated BEFORE phase A's pools so their
        # address ranges are disjoint: no release barrier between A and B.
        battn = tc.alloc_tile_pool(name="battn", bufs=2)

        utmask = cpool.tile([P, P], BF)
        ones_mat = cpool.tile([P, P], BF)

        sb_wo = wpool.tile([P, HL, HID], BF)

        # combined feature-major q+k so each chunk's RoPE output transposes
        # with a single DMA kick: groups 0..7 = q heads, 8..9 = k heads
        sb_qkT = qkvpool.tile([P, HL + KVL, S], BF)
        sb_qT = sb_qkT[:, 0:HL, :]
        sb_kT = sb_qkT[:, HL:HL + KVL, :]
        sb_v = qkvpool.tile([P, SC, KD], BF)      # token-major v
        sb_attnT = qkvpool.tile([P, HL, S], BF)   # feature-major attn out

        # ---------------- Phase A: projections + RoPE -----------------
        projpool = tc.alloc_tile_pool(name="proj", bufs=1)
        ropepool = tc.alloc_tile_pool(name="rope", bufs=2)
        # split projection PSUM: q-part [P,1024] (2 banks) x3, kv-part
        # [P,512] (1 bank) x2 -> 8 banks, three chunks in flight at ramp
        ps_qq = tc.alloc_tile_pool(name="ps_qq", bufs=3, space="PSUM")
        ps_kv = tc.alloc_tile_pool(name="ps_kv", bufs=2, space="PSUM")

        sb_xT = projpool.tile([P, KC, S], BF)
        sb_wqkv = projpool.tile([P, KC, QKVD], BF)

        xT_r = xT.rearrange("(c p) s -> p c s", p=P)
        wqkv_r = wqkv.rearrange("(c p) n -> p c n", p=P)
        # chunk 0 split fine so the very first matmul only waits on ~230 KB
        nc.sync.dma_start(out=sb_wqkv[:, 0, 0:512], in_=wqkv_r[:, 0, 0:512])
        nc.sync.dma_start(out=sb_xT[:, 0, 0:384], in_=xT_r[:, 0, 0:384])
        nc.sync.dma_start(out=sb_wqkv[:, 0, 512:QKVD],
                          in_=wqkv_r[:, 0, 512:QKVD])
        nc.sync.dma_start(out=sb_xT[:, 0, 384:S], in_=xT_r[:, 0, 384:S])
        nc.sync.dma_start(out=sb_wqkv[:, 1, :], in_=wqkv_r[:, 1, :])
        nc.sync.dma_start(out=sb_xT[:, 1, :], in_=xT_r[:, 1, :])
        sb_ck = projpool.tile([P, SC, D], BF)
        sb_sk = projpool.tile([P, SC, D], BF)
        for c in range(2, KC):
            nc.sync.dma_start(out=sb_wqkv[:, c, :], in_=wqkv_r[:, c, :])
            nc.sync.dma_start(out=sb_xT[:, c, :], in_=xT_r[:, c, :])
            if c == 5:
                # cos/sin ride mid-stream: off the ramp-critical prefix but
                # well before the first RoPE needs them
                nc.sync.dma_start(out=sb_ck[:, :, :], in_=cos_t[:, :])
                nc.sync.dma_start(out=sb_sk[:, :, :], in_=sin_t[:, :])
        # wo is only needed in phase C: delay its (4 MB) load until the
        # input streaming has drained (dep added below)
        wo_dma = nc.sync.dma_start(
            out=sb_wo[:, :, :],
            in_=wo.rearrange("(c p) n -> p c n", p=P))

        # mask/ones builders issued after the DMA starts so the sync engine
        # kicks off the input stream first (they are not needed until B)
        make_upper_triangular(nc, utmask[:, :], val=1.0, diag=True)
        nc.vector.memset(ones_mat[:, :], 1.0)

        HALF = D // 2

        def rope_block(sb_src, lo, nh, m):
            """RoPE `nh` consecutive heads of the staged bf16 chunk (cols
            [lo, lo+nh*D)) in one batched op per step, via free-dim-broadcast
            cos/sin APs.  All-bf16 so the DVE runs in 2x mode.  Returns a
            bf16 SBUF tile [P, nh*D]."""
            # t1 is produced+consumed inside one in-order DVE chain: bufs=1
            t1 = ropepool.tile([P, nh, D], BF, tag="t1", bufs=1)
            ro = ropepool.tile([P, nh * D], BF, tag="ro", bufs=2)
            src = sb_src[:, lo:lo + nh * D].rearrange("p (h d) -> p h d", h=nh)
            sin_lo = sb_sk[:, m:m + 1, 0:HALF].broadcast_to([P, nh, HALF])
            sin_hi = sb_sk[:, m:m + 1, HALF:D].broadcast_to([P, nh, HALF])
            cos_b = sb_ck[:, m:m + 1, :].broadcast_to([P, nh, D])
            # rot_half * sin (sin table pre-negated on first half)
            nc.vector.tensor_mul(t1[:, :, 0:HALF], src[:, :, HALF:D], sin_lo)
            nc.vector.tensor_mul(t1[:, :, HALF:D], src[:, :, 0:HALF], sin_hi)
            ror = ro[:, :].rearrange("p (h d) -> p h d", h=nh)
            # ro = src*cos + t1
            nc.vector.tensor_mul(ror, src, cos_b)
            nc.vector.tensor_add(ror, ror, t1[:, :, :])
            return ro

        def proj_q(pq, m, k):
            st, sp = (k == 0), (k == KC - 1)
            lhsT = sb_xT[:, k, m * P:(m + 1) * P]
            for n in (0, 1):
                mm = nc.tensor.matmul(
                    pq[:, n * 512:(n + 1) * 512], lhsT,
                    sb_wqkv[:, k, n * 512:(n + 1) * 512],
                    start=st, stop=sp)
            return mm

        def proj_kv(pkv, m, k):
            st, sp = (k == 0), (k == KC - 1)
            lhsT = sb_xT[:, k, m * P:(m + 1) * P]
            return nc.tensor.matmul(
                pkv[:, :], lhsT, sb_wqkv[:, k, 1024:QKVD],
                start=st, stop=sp)

        def stage_q(pq):
            sb_qk = ropepool.tile([P, QD + KD], BF, tag="qk")
            nc.scalar.copy(sb_qk[:, 0:512], pq[:, 0:512])
            nc.scalar.copy(sb_qk[:, 512:QD], pq[:, 512:QD])
            return sb_qk

        def stage_kv(sb_qk, pkv, m):
            nc.scalar.copy(sb_qk[:, QD:QD + KD], pkv[:, 0:KD])
            nc.scalar.copy(sb_v[:, m, :], pkv[:, KD:2 * KD])

        def finish_m(pq, pkv, m):
            # combined q+k RoPE (one 4-op DVE chain over 10 head-groups)
            # and ONE transpose kick per chunk, on the ScalarE (Act) HWDGE
            # ring so it never FIFOs behind the bulk input stream
            sb_qk = stage_q(pq)
            stage_kv(sb_qk, pkv, m)
            ms = slice(m * P, (m + 1) * P)
            qk_ro = rope_block(sb_qk, 0, HL + KVL, m)
            nc.scalar.dma_start_transpose(out=sb_qkT[:, :, ms], in_=qk_ro[:, :])

        # m=0, m=1 and m=2's q columns share each arriving k-chunk during
        # the DMA ramp (three chunks in flight across the split PSUM pools)
        pq0 = ps_qq.tile([P, QD], F32, tag="pq")
        pkv0 = ps_kv.tile([P, 2 * KD], F32, tag="pkv")
        pq1 = ps_qq.tile([P, QD], F32, tag="pq")
        pkv1 = ps_kv.tile([P, 2 * KD], F32, tag="pkv")
        pq2 = ps_qq.tile([P, QD], F32, tag="pq")
        for k in range(KC):
            proj_q(pq0, 0, k)
            proj_kv(pkv0, 0, k)
            proj_q(pq1, 1, k)
            proj_kv(pkv1, 1, k)
            mm = proj_q(pq2, 2, k)
        # release the wo load only once the input streaming has drained
        _add_dep_helper(wo_dma.ins, mm.ins,
                        reason="delay wo load past input ramp")
        finish_m(pq0, pkv0, 0)
        pkv2 = ps_kv.tile([P, 2 * KD], F32, tag="pkv")
        for k in range(KC):
            proj_kv(pkv2, 2, k)
        finish_m(pq1, pkv1, 1)
        prev = (pq2, pkv2, 2)
        for m in range(3, SC - 1):
            pq = ps_qq.tile([P, QD], F32, tag="pq")
            pkv = ps_kv.tile([P, 2 * KD], F32, tag="pkv")
            # q columns for all k first, kv columns after: the kv tile's
            # slot WAR (stage_kv two chunks back) has drained by then
            for k in range(KC):
                proj_q(pq, m, k)
            for k in range(KC):
                proj_kv(pkv, m, k)
            finish_m(*prev)
            prev = (pq, pkv, m)

        # Last chunk: q columns first so its qT (needed by phase B almost
        # immediately) lands while the PE still has kv-proj + early score
        # work; k/v columns follow and kT rides the sync ring in parallel.
        M7 = SC - 1
        pq7 = ps_qq.tile([P, QD], F32, tag="pq")
        for k in range(KC):
            proj_q(pq7, M7, k)
        finish_m(*prev)
        sb_qk7 = stage_q(pq7)
        ms7 = slice(M7 * P, (M7 + 1) * P)
        q7_ro = rope_block(sb_qk7, 0, HL, M7)
        # both last-chunk transposes ride the (by now idle) sync ring so
        # the Scalar FIFO can run stage_kv(7) + the early exps immediately
        nc.sync.dma_start_transpose(out=sb_qT[:, :, ms7], in_=q7_ro[:, :])
        pkv7 = ps_kv.tile([P, 2 * KD], F32, tag="pkv")
        for k in range(KC):
            proj_kv(pkv7, M7, k)
        stage_kv(sb_qk7, pkv7, M7)
        ps_kv.release()
        ps_qq.release()
        # Shared 4-slot [P,512] fp32 PSUM pool for score tiles, ones-matmul
        # sum tiles AND phase C's out_proj tiles; 2-slot [P,S] pool for the
        # paired heads' AV accumulators.
        ps_small = tc.alloc_tile_pool(name="ps_small", bufs=4, space="PSUM")
        ps_av = tc.alloc_tile_pool(name="ps_av", bufs=2, space="PSUM")

        exp_scale = float(1 / math.sqrt(D))
        # ragged probsT: row ki only stores the causal columns [ki*P, S)
        OFF = [0]
        for ki in range(SC):
            OFF.append(OFF[-1] + S - ki * P)
        PTOT = OFF[-1]          # 4608

        head_tiles = {}

        def get_head_tiles(h):
            if h not in head_tiles:
                probsT = battn.tile([P, PTOT], BF, tag="probsT", bufs=3,
                                    name=f"probsT{h}")
                acc = battn.tile([P, S], BF, tag="acc", bufs=3,
                                 name=f"acc{h}")
                head_tiles[h] = (probsT, acc)
            return head_tiles[h]

        def pt(probsT, ki, a, b):
            q0 = ki * P
            return probsT[:, OFF[ki] + (a - q0):OFF[ki] + (b - q0)]

        # early score pieces: cover the PE while the last chunk's RoPE +
        # transposes drain (they only touch token chunks 0-3)
        early_done = set()
        for h in (0, 1, 2):
            probsT, _ = get_head_tiles(h)
            for ki in range(4):
                a = ki * P
                psc = ps_small.tile([P, 512], F32, tag="ps", name="psce")
                nc.tensor.matmul(psc[:, 0:512 - a], sb_kT[:, 0, a:a + P],
                                 sb_qT[:, h, a:512], start=True, stop=True)
                nc.scalar.activation(pt(probsT, ki, a, 512),
                                     psc[:, 0:512 - a],
                                     Exp, scale=exp_scale)
                early_done.add((h, ki, a))

        k7_ro = rope_block(sb_qk7, QD, KVL, M7)
        nc.sync.dma_start_transpose(out=sb_kT[:, :, ms7], in_=k7_ro[:, :])
        ropepool.release()
        projpool.release()
        # out_proj partials for heads 0-3 (computed during B pairs 2-3)
        # live in the space the projection pools just freed
        partpool = tc.alloc_tile_pool(name="part", bufs=1)
        sb_part = partpool.tile([P, SC, HID], BF)

        # ---------------- Phase B: causal attention -------------------
        # Heads processed in pairs with interleaved k-chunks: one head's
        # exp/mask/acc chain hides under the other's score/AV matmuls.

        def head_ctx(h):
            g = h // (HL // KVL)
            probsT, acc = get_head_tiles(h)
            pav = ps_av.tile([P, S], F32, tag="pav")
            return (h, g, probsT, acc, pav)

        def pieces_ki(ctx, ki):
            h, g, probsT, acc, pav = ctx
            q0 = ki * P
            kslice = slice(q0, q0 + P)
            for (a, b) in _pieces(q0, S):
                if (h, ki, a) in early_done:
                    continue
                psc = ps_small.tile([P, 512], F32, tag="ps")
                nc.tensor.matmul(psc[:, 0:b - a],
                                 sb_kT[:, g, kslice],
                                 sb_qT[:, h, a:b],
                                 start=True, stop=True)
                nc.scalar.activation(pt(probsT, ki, a, b),
                                     psc[:, 0:b - a], Exp,
                                     scale=exp_scale)
            # mask strictly-below-diagonal of the diag block on GpSimd
            nc.gpsimd.tensor_mul(pt(probsT, ki, q0, q0 + P),
                                 pt(probsT, ki, q0, q0 + P),
                                 utmask[:, :])
            # accumulate the column sums on DVE (2x bf16)
            if ki == 0:
                nc.vector.tensor_copy(acc[:, :], pt(probsT, 0, 0, S))
            else:
                nc.vector.tensor_add(acc[:, q0:], acc[:, q0:],
                                     pt(probsT, ki, q0, S))

        def av_ki(ctx, ki):
            h, g, probsT, acc, pav = ctx
            st, sp = (ki == 0), (ki == SC - 1)
            for (a, b) in _pieces(ki * P, S):
                nc.tensor.matmul(pav[:, a:b],
                                 sb_v[:, ki, g * D:(g + 1) * D],
                                 pt(probsT, ki, a, b),
                                 start=st, stop=sp)

        def finalize_pair(cA, cB):
            # PE stream stays dense: both heads' last AV, then both ones
            # passes, with the DVE recip/mul tails trailing
            av_ki(cA, SC - 1)
            av_ki(cB, SC - 1)
            for ctx, rb in ((cA, 0), (cB, 1)):
                h, g, probsT, acc, pav = ctx
                rbc = battn.tile([P, S], F32, tag="rbc", bufs=1)
                for (a, b) in _pieces(0, S):
                    psbc = ps_small.tile([P, 512], F32, tag="ps")
                    nc.tensor.matmul(psbc[:, 0:b - a], ones_mat[:, :],
                                     acc[:, a:b], start=True, stop=True)
                    nc.vector.reciprocal_approx_fast(rbc[:, a:b],
                                                     psbc[:, 0:b - a])
                nc.vector.tensor_mul(sb_attnT[:, h, :], pav[:, :], rbc[:, :])

        def partial_block(m, nb):
            # out_proj contribution of heads 0-3 for block (m, nb), staged
            # to SBUF bf16: fills the PE while ScalarE paces the exp chain
            ms = slice(m * P, (m + 1) * P)
            nsl = slice(nb * 512, (nb + 1) * 512)
            py = ps_small.tile([P, 512], F32, tag="ps")
            for k in range(4):
                nc.tensor.matmul(py[:, :], sb_attnT[:, k, ms],
                                 sb_wo[:, k, nsl],
                                 start=(k == 0), stop=(k == 3))
            if (m + nb) % 2:
                nc.vector.tensor_copy(sb_part[:, m, nsl], py[:, :])
            else:
                nc.scalar.copy(sb_part[:, m, nsl], py[:, :])

        part_iter = iter([(m, nb) for m in range(SC)
                          for nb in range(HID // 512)])
        pending = [None]
        for hp in range(HL // 2):
            if pending[0] is not None:
                # finalize the previous pair before its pav slots rotate
                pending[0]()
                pending[0] = None
            ctxA = head_ctx(2 * hp)
            ctxB = head_ctx(2 * hp + 1)
            for ki in range(SC):
                pieces_ki(ctxA, ki)
                pieces_ki(ctxB, ki)
                if ki >= 1:
                    av_ki(ctxA, ki - 1)
                    av_ki(ctxB, ki - 1)
                if hp >= 2:
                    partial_block(*next(part_iter))
                    partial_block(*next(part_iter))

            def make_pending(cA, cB):
                def run():
                    finalize_pair(cA, cB)
                return run
            pending[0] = make_pending(ctxA, ctxB)

        # ---------------- Phase C: out projection ---------------------
        # Fine-grained: one 512-col PSUM slot per n-block with k-inner
        # accumulation.  m=0's first blocks run k<6 while the last pair's
        # finalize chains (attnT[6], attnT[7]) drain.
        ypool = tc.alloc_tile_pool(name="ysb", bufs=2)
        # the last pair finalizes first (before any phase-C ps_small allocs
        # so its ones-tiles don't rotate onto a held out_proj slot); m=0's
        # first blocks then run k=4,5 while attnT[6]/attnT[7] drain
        pending[0]()
        for m in range(SC):
            ms = slice(m * P, (m + 1) * P)
            last_m = (m == SC - 1)
            ysb = ypool.tile([P, HID], BF, tag="ysb")
            pys = {}
            for nb in range(HID // 512):
                nsl = slice(nb * 512, (nb + 1) * 512)
                py = ps_small.tile([P, 512], F32, tag="ps")
                if m == 0 and nb < 2:
                    pys[nb] = py
                    for k in (4, 5):
                        nc.tensor.matmul(py[:, :], sb_attnT[:, k, ms],
                                         sb_wo[:, k, nsl],
                                         start=(k == 4), stop=False)
                    if nb == 0:
                        continue
                    for pnb in (0, 1):
                        pnsl = slice(pnb * 512, (pnb + 1) * 512)
                        for k in (HL - 2, HL - 1):
                            nc.tensor.matmul(pys[pnb][:, :],
                                             sb_attnT[:, k, ms],
                                             sb_wo[:, k, pnsl],
                                             start=False, stop=(k == HL - 1))
                    nc.vector.tensor_add(ysb[:, 0:512], pys[0][:, :],
                                         sb_part[:, 0, 0:512])
                else:
                    for k in range(4, HL):
                        nc.tensor.matmul(py[:, :],
                                         sb_attnT[:, k, ms],
                                         sb_wo[:, k, nsl],
                                         start=(k == 4), stop=(k == HL - 1))
                # combine the heads 4-7 PSUM block with the heads 0-3
                # bf16 partial on the DVE
                if last_m and nb == 3:
                    # tail: combine+store the final block in 256-col halves
                    # so the very last DMA only moves 64KB
                    for h0, h1 in ((1536, 1792), (1792, 2048)):
                        nc.vector.tensor_add(ysb[:, h0:h1],
                                             py[:, h0 - 1536:h1 - 1536],
                                             sb_part[:, m, h0:h1])
                        nc.sync.dma_start(out=out[ms, h0:h1],
                                          in_=ysb[:, h0:h1])
                    continue
                nc.vector.tensor_add(ysb[:, nsl], py[:, :],
                                     sb_part[:, m, nsl])
                if nb % 2 == 1:
                    if not last_m:
                        # store per 1024-col pair
                        nc.sync.dma_start(
                            out=out[ms, nb * 512 - 512:nb * 512 + 512],
                            in_=ysb[:, nb * 512 - 512:nb * 512 + 512])
                    else:
                        nc.sync.dma_start(out=out[ms, 0:1024],
                                          in_=ysb[:, 0:1024])
                elif last_m and nb == 2:
                    # tail: store this block early so the final kick
                    # only moves the last 128KB
                    nc.sync.dma_start(out=out[ms, 1024:1536],
                                      in_=ysb[:, 1024:1536])

        ypool.release()
        partpool.release()
        ps_av.release()
        ps_small.release()
        battn.release()
        qkvpool.release()
        wpool.release()
        cpool.release()

    nc.compile()
    return nc


def _get_nc():
    if "nc" not in _NC_CACHE:
        _NC_CACHE["nc"] = build_nc()
    return _NC_CACHE["nc"]


def _chunk_major(a, nchunks):
    """[nchunks*128, cols] -> [128, nchunks*cols] partition-major layout."""
    n = a.shape[1]
    return np.ascontiguousarray(
        a.reshape(nchunks, P, n).transpose(1, 0, 2).reshape(P, nchunks * n))


def _make_in_maps(x, cos, sin, wq, wk, wv, wo):
    bf = ml_dtypes.bfloat16
    HALF = D // 2
    sin_rot = np.concatenate([-sin[:, :HALF], sin[:, HALF:]], axis=1)
    cos_t = _chunk_major(cos, SC).astype(bf)
    sin_t = _chunk_major(sin_rot, SC).astype(bf)
    in_maps = []
    for core in range(NCORES):
        b, t = divmod(core, TP)
        wqkv = np.concatenate([
            wq[:, t * QD:(t + 1) * QD],
            wk[:, t * KD:(t + 1) * KD],
            wv[:, t * KD:(t + 1) * KD],
        ], axis=1)
        in_maps.append({
            "xT": np.ascontiguousarray(x[b].T).astype(bf),
            "wqkv": np.ascontiguousarray(wqkv).astype(bf),
            "wo": np.ascontiguousarray(wo[t * QD:(t + 1) * QD, :]).astype(bf),
            "cos_t": cos_t, "sin_t": sin_t,
        })
    return in_maps


def run(inputs, trace=False):
    if trace:
        _ensure_ntff_hook()
    nc = _get_nc()
    in_maps = _make_in_maps(
        np.asarray(inputs["x"], np.float32),
        np.asarray(inputs["cos"], np.float32),
        np.asarray(inputs["sin"], np.float32),
        np.asarray(inputs["wq"], np.float32),
        np.asarray(inputs["wk"], np.float32),
        np.asarray(inputs["wv"], np.float32),
        np.asarray(inputs["wo"], np.float32),
    )
    try:
        res = run_bass_kernel_spmd(nc, in_maps, list(range(NCORES)),
                                   trace=trace)
    except Exception:
        # one retry: a previous process can leave a core wedged
        res = run_bass_kernel_spmd(nc, in_maps, list(range(NCORES)),
                                   trace=trace)
    outs = [np.asarray(r["out"]).astype(np.float32) for r in res.results]
    y = np.stack([outs[TP * b] + outs[TP * b + 1] for b in range(B)])
    return y, res


def kernel(**inputs):
    y, _ = run(inputs, trace=False)
    return y
